# revision 1
# baseline (speedup 1.0000x reference)
"""Trainium2 Bass kernel for nn_Attention (B=2, C=256, H=W=64, 8 heads).

Sharding: 8 cores = 2 batches x 4 query-chunks (1024 queries each), no
collectives. Each core gets its batch's full x with token columns rolled so
its own query chunk sits at columns 0:1024 (attention is permutation-
invariant over keys); it computes LN + projections + attention for its
queries and writes a [256, 1024] slice of the output.

Everything stays in the transposed [channel, token] layout (x's native DRAM
layout): LN (stats via ones-matmul over the partition dim) -> qT/kT
projections -> S^T = K Q^T (K=32 matmuls packed 4-way into PE row groups)
-> exp -> P^T V via stationary-V matmuls with an appended ones column
(softmax denominators for free) -> normalize -> out-projection -> residual.

exp is split between ScalarE (true exp) and VectorE (Schraudolph: Wq is
pre-scaled so PSUM = 128*log2(e)*logit; adding a magic bias and converting
f32->int16 yields the bf16 bit pattern of 2^y, ~3% max rel err, harmless
here because the attention branch contributes ~0.2% of the output next to
the residual).
"""

import numpy as np

B, C, H, W = 2, 256, 64, 64
N = H * W            # 4096 tokens
NH, HD = 8, 32       # heads, head_dim
NQ = N // 4          # queries per core
LN_EPS = 1e-5
LOG2E = 1.4426950408889634
LN2 = 0.6931471805599453
ATTN_SCALE = HD ** -0.5
A_SCALE = 128.0 * LOG2E * ATTN_SCALE   # folded into Wq on host
B16F = 16256.0 - 5.6                   # Schraudolph bias (calibrated)
ACT_EXP_SHARE = 5                      # j%8 < ACT_EXP_SHARE -> ScalarE exp

_PROFILE = False
_CACHE = {}


def _build():
    from concourse import bacc
    from concourse import mybir
    import concourse.tile as tile
    import dataclasses

    f32 = mybir.dt.float32
    bf16 = mybir.dt.bfloat16
    i16 = mybir.dt.int16
    ALU = mybir.AluOpType
    ACTF = mybir.ActivationFunctionType

    nc = bacc.Bacc("TRN2", target_bir_lowering=False)
    xd = nc.dram_tensor("x", [C, N], f32, kind="ExternalInput")
    wq = nc.dram_tensor("wqT", [C, C], bf16, kind="ExternalInput")  # pre-scaled
    wk = nc.dram_tensor("wkT", [C, C], bf16, kind="ExternalInput")
    wv = nc.dram_tensor("wvT", [C, C], bf16, kind="ExternalInput")
    wp = nc.dram_tensor("wpT", [C, C], bf16, kind="ExternalInput")
    gam = nc.dram_tensor("gam", [C, 1], f32, kind="ExternalInput")
    bet = nc.dram_tensor("bet", [C, 1], f32, kind="ExternalInput")
    bpd = nc.dram_tensor("bp", [C, 1], f32, kind="ExternalInput")
    od = nc.dram_tensor("out", [C, NQ], f32, kind="ExternalOutput")

    def bcast(ap, parts):
        # replicate one partition across `parts` partitions (DMA source only)
        return dataclasses.replace(ap, ap=[[0, parts]] + list(ap.ap[1:]))

    with tile.TileContext(nc) as tc:
        with tc.tile_pool(name="big", bufs=1) as big, \
             tc.tile_pool(name="sml", bufs=4) as sml:

            # ---- load inputs ----
            x_sb = [big.tile([128, N], f32, tag=f"x{c}", name=f"x{c}") for c in range(2)]
            for c in range(2):
                nc.sync.dma_start(out=x_sb[c][:, :], in_=xd[c * 128:(c + 1) * 128, :])
            w_sb = {}
            for name, t in (("q", wq), ("k", wk), ("v", wv), ("p", wp)):
                for c in range(2):
                    s = big.tile([128, C], bf16, tag=f"w{name}{c}", name=f"w{name}{c}")
                    nc.sync.dma_start(out=s[:, :], in_=t[c * 128:(c + 1) * 128, :])
                    w_sb[name, c] = s
            gam_sb = [big.tile([128, 1], f32, tag=f"g{c}", name=f"g{c}") for c in range(2)]
            bet_sb = [big.tile([128, 1], f32, tag=f"b{c}", name=f"b{c}") for c in range(2)]
            bp_sb = [big.tile([128, 1], f32, tag=f"bp{c}", name=f"bp{c}") for c in range(2)]
            for c in range(2):
                nc.sync.dma_start(out=gam_sb[c][:, :], in_=gam[c * 128:(c + 1) * 128, :])
                nc.sync.dma_start(out=bet_sb[c][:, :], in_=bet[c * 128:(c + 1) * 128, :])
                nc.sync.dma_start(out=bp_sb[c][:, :], in_=bpd[c * 128:(c + 1) * 128, :])
            ones_sb = big.tile([128, 1], f32, tag="ones", name="ones")
            nc.vector.memset(ones_sb[:, :], 1.0 / C)
            ones_row = big.tile([1, 128], f32, tag="onesr", name="onesr")
            nc.vector.memset(ones_row[:, :], 1.0)

            tn = [big.tile([128, N], bf16, tag=f"tn{c}", name=f"tn{c}") for c in range(2)]

            # ---- LayerNorm ----
            with tc.tile_pool(name="lnp", bufs=1) as lnp, \
                 tc.tile_pool(name="lns", bufs=2, space="PSUM") as lns:
                sq = [lnp.tile([128, N], f32, tag=f"sq{c}", name=f"sq{c}") for c in range(2)]
                for c in range(2):
                    nc.scalar.activation(sq[c][:, :], x_sb[c][:, :], ACTF.Square)
                eps_sc = lnp.tile([1, 1], f32, tag="epssc", name="epssc")
                nc.vector.memset(eps_sc[:, :], LN_EPS)
                for f in range(8):
                    sl = slice(f * 512, (f + 1) * 512)
                    mps = lns.tile([1, 512], f32, tag="mps", name="mps")
                    nc.tensor.matmul(mps[:, :], ones_sb[:, :], x_sb[0][:, sl], start=True, stop=False)
                    nc.tensor.matmul(mps[:, :], ones_sb[:, :], x_sb[1][:, sl], start=False, stop=True)
                    mu_sb = sml.tile([1, 512], f32, tag="musb", name="musb")
                    nc.vector.tensor_copy(mu_sb[:, :], mps[:, :])
                    eps_t = lns.tile([1, 512], f32, tag="eps", name="eps")
                    nc.tensor.matmul(eps_t[:, :], ones_sb[:, :], sq[0][:, sl], start=True, stop=False)
                    nc.tensor.matmul(eps_t[:, :], ones_sb[:, :], sq[1][:, sl], start=False, stop=True)
                    var_sb = sml.tile([1, 512], f32, tag="varsb", name="varsb")
                    nc.vector.tensor_tensor(var_sb[:, :], mu_sb[:, :], mu_sb[:, :], ALU.mult)
                    nc.vector.tensor_tensor(var_sb[:, :], eps_t[:, :], var_sb[:, :], ALU.subtract)
                    std_sb = sml.tile([1, 512], f32, tag="stdsb", name="stdsb")
                    nc.scalar.activation(std_sb[:, :], var_sb[:, :], ACTF.Sqrt, bias=eps_sc[:, :])
                    rs_sb = sml.tile([1, 512], f32, tag="rssb", name="rssb")
                    nc.vector.reciprocal(rs_sb[:, :], std_sb[:, :])
                    mu_b = lns.tile([128, 512], f32, tag="mub", name="mub")
                    rs_b = lns.tile([128, 512], f32, tag="rsb", name="rsb")
                    nc.tensor.matmul(mu_b[:, :], ones_row[:, :], mu_sb[:, :],
                                     start=True, stop=True, tile_position=(0, 0))
                    nc.tensor.matmul(rs_b[:, :], ones_row[:, :], rs_sb[:, :],
                                     start=True, stop=True, tile_position=(0, 0))
                    for c in range(2):
                        t = lnp.tile([128, 512], f32, tag=f"t{c}", name=f"t{c}")
                        nc.vector.tensor_tensor(t[:, :], x_sb[c][:, sl], mu_b[:, :], ALU.subtract)
                        nc.vector.tensor_tensor(t[:, :], t[:, :], rs_b[:, :], ALU.mult)
                        nc.vector.tensor_scalar(tn[c][:, sl], t[:, :], gam_sb[c][:, :],
                                                bet_sb[c][:, :], ALU.mult, ALU.add)

            # ---- q/k/v projections ----
            qT = [big.tile([128, NQ], bf16, tag=f"qT{c}", name=f"qT{c}") for c in range(2)]
            kT = [big.tile([128, N], bf16, tag=f"kT{c}", name=f"kT{c}") for c in range(2)]
            v_sb = big.tile([128, 32, NH, 33], bf16, tag="v", name="v")
            nc.vector.memset(v_sb[:, :, :, 32:33], 1.0)
            with tc.tile_pool(name="mm", bufs=2, space="PSUM") as mmp:
                for co in range(2):
                    for f in range(N // 512):
                        sl = slice(f * 512, (f + 1) * 512)
                        ps = mmp.tile([128, 512], f32, tag="proj", name="proj")
                        for ci in range(2):
                            nc.tensor.matmul(ps[:, :], w_sb["k", ci][:, co * 128:(co + 1) * 128],
                                             tn[ci][:, sl], start=(ci == 0), stop=(ci == 1))
                        nc.scalar.copy(kT[co][:, sl], ps[:, :])
                    for f in range(NQ // 512):
                        sl = slice(f * 512, (f + 1) * 512)
                        ps = mmp.tile([128, 512], f32, tag="proj", name="proj")
                        for ci in range(2):
                            nc.tensor.matmul(ps[:, :], w_sb["q", ci][:, co * 128:(co + 1) * 128],
                                             tn[ci][:, sl], start=(ci == 0), stop=(ci == 1))
                        nc.scalar.copy(qT[co][:, sl], ps[:, :])
                for j in range(32):
                    jl = slice(j * 128, (j + 1) * 128)
                    ps = mmp.tile([128, 256], f32, tag="vproj", name="vproj")
                    for ci in range(2):
                        nc.tensor.matmul(ps[:, :], tn[ci][:, jl], w_sb["v", ci][:, :],
                                         start=(ci == 0), stop=(ci == 1))
                    nc.vector.tensor_copy(v_sb[:, j, :, 0:32],
                                          ps[:, :].rearrange("p (h d) -> p h d", h=NH))

            # ---- attention ----
            attnT = [big.tile([128, NQ], bf16, tag=f"at{c}", name=f"at{c}") for c in range(2)]
            with tc.tile_pool(name="sps", bufs=2, space="PSUM") as sp, \
                 tc.tile_pool(name="avp", bufs=1, space="PSUM") as avp, \
                 tc.tile_pool(name="xtr", bufs=1, space="PSUM") as xtr, \
                 tc.tile_pool(name="warm", bufs=1, space="PSUM") as warmp, \
                 tc.tile_pool(name="pp", bufs=3) as ppool, \
                 tc.tile_pool(name="nrm", bufs=4) as nrm:
                for f in range(NQ // 512):
                    fl = slice(f * 512, (f + 1) * 512)
                    for hg in range(2):
                        av = [avp.tile([128, 512], f32, tag=f"av{pr}", name=f"av{pr}") for pr in range(2)]
                        for j in range(32):
                            jl = slice(j * 128, (j + 1) * 128)
                            # full-array matmul so the HAM clock-gate sees PE
                            # activity (masked tile_position matmuls don't
                            # count) and keeps the 2.4 GHz clock
                            wps = warmp.tile([128, 64], f32, tag="warm", name="warm")
                            nc.tensor.matmul(wps[:, :], w_sb["p", 0][:, 0:128],
                                             tn[0][:, 0:64], start=True, stop=True)
                            ss = [sp.tile([128, 512], f32, tag=f"s{i % 2}", name=f"s{i % 2}") for i in range(4)]
                            pt = [ppool.tile([128, 512], bf16, tag=f"p{i}", name=f"p{i}") for i in range(4)]
                            for i in range(4):
                                rr = slice(i * 32, (i + 1) * 32)
                                nc.tensor.matmul(ss[i][:, :], kT[hg][rr, jl], qT[hg][rr, fl],
                                                 start=True, stop=True,
                                                 tile_position=(i * 32, 0))
                            for i in range(4):
                                if j % 8 < ACT_EXP_SHARE:
                                    nc.scalar.activation(pt[i][:, :], ss[i][:, :],
                                                         ACTF.Exp, scale=LN2 / 128.0)
                                else:
                                    nc.vector.tensor_scalar(
                                        pt[i][:, :].bitcast(i16), ss[i][:, :],
                                        B16F, None, ALU.add)
                            for pr in range(2):
                                for t2 in range(2):
                                    h = pr * 2 + t2
                                    nc.tensor.matmul(
                                        av[pr][t2 * 64:t2 * 64 + 33, :],
                                        v_sb[:, j, hg * 4 + h, :], pt[h][:, :],
                                        start=(j == 0), stop=(j == 31),
                                        tile_position=(0, t2 * 64))
                        for pr in range(2):
                            for t2 in range(2):
                                rbase = t2 * 64
                                rcp = nrm.tile([1, 512], f32, tag=f"rc{pr}{t2}", name=f"rc{pr}{t2}")
                                nc.vector.reciprocal(rcp[:, :], av[pr][rbase + 32:rbase + 33, :])
                                bc = xtr.tile([32, 512], f32, tag="bc", name="bc")
                                nc.tensor.matmul(bc[:, :], ones_row[:, 0:32], rcp[:, :],
                                                 start=True, stop=True)
                                bcs = nrm.tile([32, 512], f32, tag="bcs", name="bcs")
                                nc.vector.tensor_copy(bcs[:, :], bc[:, :])
                                row0 = (pr * 2 + t2) * 32
                                nc.vector.tensor_tensor(
                                    attnT[hg][row0:row0 + 32, fl],
                                    av[pr][rbase:rbase + 32, :], bcs[:, :], ALU.mult)

            # ---- output projection + residual ----
            with tc.tile_pool(name="mm2", bufs=2, space="PSUM") as mm2, \
                 tc.tile_pool(name="ot", bufs=4) as otp:
                for mo in range(2):
                    for f in range(NQ // 512):
                        sl = slice(f * 512, (f + 1) * 512)
                        ps = mm2.tile([128, 512], f32, tag="o", name="o")
                        for ci in range(2):
                            nc.tensor.matmul(ps[:, :], w_sb["p", ci][:, mo * 128:(mo + 1) * 128],
                                             attnT[ci][:, sl], start=(ci == 0), stop=(ci == 1))
                        ot = otp.tile([128, 512], f32, tag="ot", name="ot")
                        nc.vector.tensor_tensor(ot[:, :], ps[:, :], x_sb[mo][:, sl], ALU.add)
                        nc.vector.tensor_scalar(ot[:, :], ot[:, :], bp_sb[mo][:, :],
                                                None, ALU.add)
                        nc.sync.dma_start(out=od[mo * 128:(mo + 1) * 128, sl], in_=ot[:, :])

    nc.finalize()
    return nc


def kernel(x, ln_gamma, ln_beta, w_qkv, w_proj, b_proj):
    import ml_dtypes
    from concourse.bass_utils import run_bass_kernel_spmd

    if "nc" not in _CACHE:
        _CACHE["nc"] = _build()
    nc = _CACHE["nc"]

    x = np.asarray(x, np.float32)
    w_qkv = np.asarray(w_qkv, np.float32)
    bf = ml_dtypes.bfloat16
    wqT = np.ascontiguousarray((A_SCALE * w_qkv[0:C]).T.astype(bf))
    wkT = np.ascontiguousarray(w_qkv[C:2 * C].T.astype(bf))
    wvT = np.ascontiguousarray(w_qkv[2 * C:3 * C].T.astype(bf))
    wpT = np.ascontiguousarray(np.asarray(w_proj, np.float32).T.astype(bf))
    gam = np.asarray(ln_gamma, np.float32).reshape(C, 1)
    bet = np.asarray(ln_beta, np.float32).reshape(C, 1)
    bp = np.asarray(b_proj, np.float32).reshape(C, 1)

    xf = x.reshape(B, C, N)
    in_maps = []
    for core in range(8):
        b, qc = core // 4, core % 4
        xr = np.roll(xf[b], -qc * NQ, axis=1)
        in_maps.append({
            "x": np.ascontiguousarray(xr), "wqT": wqT, "wkT": wkT,
            "wvT": wvT, "wpT": wpT, "gam": gam, "bet": bet, "bp": bp,
        })

    res = run_bass_kernel_spmd(nc, in_maps, core_ids=list(range(8)),
                               trace=_PROFILE)
    if _PROFILE:
        _CACHE["exec_time_ns"] = res.exec_time_ns
    out = np.empty((B, C, N), np.float32)
    for core in range(8):
        b, qc = core // 4, core % 4
        out[b][:, qc * NQ:(qc + 1) * NQ] = res.results[core]["out"]
    return out.reshape(B, C, H, W)



# revision 31
# speedup vs baseline: 1.1803x; 1.1803x over previous
"""Trainium2 Bass kernel for nn_Attention (B=2, C=256, H=W=64, 8 heads).

Sharding: 8 cores = 2 batches x 4 query-chunks (1024 queries each), no
collectives. Each core gets its batch's full x (bf16) with token columns
rolled so its own query chunk sits at columns 0:1024 (attention is
permutation-invariant over keys); it computes LN + projections + attention
for its queries and writes a [256, 1024] slice of the output.

Key structure (v2 — rebuilt for PE throughput):
- LN is folded into the projections: gamma is pre-multiplied into the
  weights on the host; the per-token mean/rstd enter as a rank-1 fixup
  matmul accumulated into each projection's PSUM (lhsT = [-rowsum(W); W@beta],
  rhs = [mu; sqrt(var+eps)]) followed by a *rstd multiply at PSUM
  evacuation. No normalized-x tensor is ever materialized.
- S^T matmuls are full-array (unmasked): the stationary is the whole
  4-head K chunk [128x128]; per-head Q lives in zero-padded [128, NQ]
  tiles so each 512-query matmul contracts over all 128 channel rows but
  only the head's 32 rows are nonzero. This keeps the PE HAM clock-gate
  warm (masked tile_position matmuls don't count as PE activity) and
  shares one LDWEIGHTS across the 4 S matmuls of a key chunk.
- exp splits per key-chunk between ScalarE (true exp on head-pair 0) and
  VectorE (Schraudolph bit-trick on head-pair 1): Wq is pre-scaled so
  PSUM = 128*log2(e)*logit; adding a magic bias and converting f32->int16
  yields the bf16 bit pattern of 2^y (~3% max rel err, harmless next to
  the residual). Both pairs process [128,1024] two-bank PSUM tiles.
- P^T V via stationary-V matmuls with an appended ones column gives the
  softmax denominators for free; normalization uses the fast custom-DVE
  reciprocal (~5x faster than the 8-pass DIV) broadcast through a tiny
  f32r matmul.
"""

import numpy as np

B, C, H, W = 2, 256, 64, 64
N = H * W            # 4096 tokens
NH, HD = 8, 32       # heads, head_dim
NQ = N // 4          # queries per core
LN_EPS = 1e-5
LOG2E = 1.4426950408889634
LN2 = 0.6931471805599453
ATTN_SCALE = HD ** -0.5
A_SCALE = 128.0 * LOG2E * ATTN_SCALE   # folded into Wq on host
B16F = 16256.0 - 5.6                   # Schraudolph bias (calibrated)

_PROFILE = False
_CACHE = {}


def _build():
    from concourse import bacc
    from concourse import mybir
    import concourse.tile as tile

    f32 = mybir.dt.float32
    f32r = mybir.dt.float32r
    bf16 = mybir.dt.bfloat16
    i16 = mybir.dt.int16
    ALU = mybir.AluOpType
    ACTF = mybir.ActivationFunctionType

    nc = bacc.Bacc("TRN2", target_bir_lowering=False)
    xbd = nc.dram_tensor("xb", [C, N], bf16, kind="ExternalInput")
    xfd = nc.dram_tensor("xf", [C, NQ], f32, kind="ExternalInput")
    wq = nc.dram_tensor("wqT", [C, C], bf16, kind="ExternalInput")  # gamma+A_SCALE folded
    wk = nc.dram_tensor("wkT", [C, C], bf16, kind="ExternalInput")
    wv = nc.dram_tensor("wvT", [C, NH * 33], bf16, kind="ExternalInput")
    wp = nc.dram_tensor("wpT", [C, C], bf16, kind="ExternalInput")
    # rank-1 LN fixup rows: *0 = -rowsum(W'), *1 = W@beta (each own tensor so
    # every engine/matmul access starts at partition 0)
    wbq0 = nc.dram_tensor("wbq0", [1, C], bf16, kind="ExternalInput")
    wbq1 = nc.dram_tensor("wbq1", [1, C], bf16, kind="ExternalInput")
    wbk0 = nc.dram_tensor("wbk0", [1, C], bf16, kind="ExternalInput")
    wbk1 = nc.dram_tensor("wbk1", [1, C], bf16, kind="ExternalInput")
    wbv0 = nc.dram_tensor("wbv0", [1, NH * 33], bf16, kind="ExternalInput")
    wbv1 = nc.dram_tensor("wbv1", [1, NH * 33], bf16, kind="ExternalInput")
    bpd = nc.dram_tensor("bp", [C, 1], f32, kind="ExternalInput")
    od = nc.dram_tensor("out", [C, NQ], f32, kind="ExternalOutput")

    with tile.TileContext(nc) as tc:
        with tc.tile_pool(name="big", bufs=1) as big, \
             tc.tile_pool(name="sml", bufs=2) as sml:

            # ---- load inputs ----
            xb = [big.tile([128, N], bf16, tag=f"xb{c}", name=f"xb{c}") for c in range(2)]
            for c in range(2):
                nc.sync.dma_start(out=xb[c][:, :], in_=xbd[c * 128:(c + 1) * 128, :])
            xf = [big.tile([128, NQ], f32, tag=f"xf{c}", name=f"xf{c}") for c in range(2)]
            for c in range(2):
                nc.sync.dma_start(out=xf[c][:, :], in_=xfd[c * 128:(c + 1) * 128, :])
            w_sb = {}
            for name, t, nout in (("q", wq, C), ("k", wk, C),
                                  ("v", wv, NH * 33), ("p", wp, C)):
                for c in range(2):
                    s = big.tile([128, nout], bf16, tag=f"w{name}{c}", name=f"w{name}{c}")
                    nc.sync.dma_start(out=s[:, :], in_=t[c * 128:(c + 1) * 128, :])
                    w_sb[name, c] = s
            wb_sb = {}
            for name, t, nout in (("q0", wbq0, C), ("q1", wbq1, C),
                                  ("k0", wbk0, C), ("k1", wbk1, C),
                                  ("v0", wbv0, NH * 33), ("v1", wbv1, NH * 33)):
                s = big.tile([1, nout], bf16, tag=f"wb{name}", name=f"wb{name}")
                nc.sync.dma_start(out=s[:, :], in_=t[:, :])
                wb_sb[name] = s
            bp_sb = [big.tile([128, 1], f32, tag=f"bp{c}", name=f"bp{c}") for c in range(2)]
            for c in range(2):
                nc.sync.dma_start(out=bp_sb[c][:, :], in_=bpd[c * 128:(c + 1) * 128, :])

            onesC = big.tile([128, 1], bf16, tag="onesC", name="onesC")
            nc.vector.memset(onesC[:, :], 1.0 / C)
            ones_row = big.tile([1, 128], bf16, tag="onesr", name="onesr")
            nc.vector.memset(ones_row[:, :], 1.0)
            ident = big.tile([1, 1], f32, tag="ident", name="ident")
            nc.vector.memset(ident[:, :], 1.0)

            rs_cols = big.tile([128, 32], f32, tag="rscols", name="rscols")  # rstd, col layout

            kT = [big.tile([128, N], bf16, tag=f"kT{c}", name=f"kT{c}") for c in range(2)]
            qp = [[big.tile([128, NQ], bf16, tag=f"qp{hg}{h}", name=f"qp{hg}{h}")
                   for h in range(4)] for hg in range(2)]
            for hg in range(2):
                for h in range(4):
                    nc.gpsimd.memset(qp[hg][h][:, :], 0.0)
            v_sb = big.tile([128, 32, NH, 33], bf16, tag="v", name="v")
            attnT = [big.tile([128, NQ], bf16, tag=f"at{c}", name=f"at{c}") for c in range(2)]

            # ---- LN stats + projections ----
            with tc.tile_pool(name="lnsb", bufs=1) as lnsb, \
                 tc.tile_pool(name="lnp", bufs=1, space="PSUM") as lnp, \
                 tc.tile_pool(name="mm", bufs=2, space="PSUM") as mmp:
                mu_row = lnsb.tile([1, N], bf16, tag="murow", name="murow")
                srt_row = lnsb.tile([1, N], bf16, tag="srtrow", name="srtrow")
                rs_row = lnsb.tile([1, N], f32, tag="rsrow", name="rsrow")
                rs_bf = lnsb.tile([1, N], bf16, tag="rsbf", name="rsbf")
                rs_ball = lnsb.tile([128, N], f32, tag="rsball", name="rsball")
                xsq = [lnsb.tile([128, N], bf16, tag=f"xsq{c}", name=f"xsq{c}") for c in range(2)]
                nc.vector.tensor_tensor(xsq[0][:, :], xb[0][:, :], xb[0][:, :], ALU.mult)
                nc.vector.tensor_tensor(xsq[1][:, :], xb[1][:, :], xb[1][:, :], ALU.mult)
                rsT_ps = lnp.tile([128, 32], f32, tag="rsT", name="rsT")
                for f in range(8):
                    fl = slice(f * 512, (f + 1) * 512)
                    mps = lnp.tile([1, 512], f32, tag="mps", name="mps")
                    nc.tensor.matmul(mps[:, :], onesC[:, :], xb[0][:, fl], start=True, stop=False)
                    nc.tensor.matmul(mps[:, :], onesC[:, :], xb[1][:, fl], start=False, stop=True)
                    sps = lnp.tile([1, 512], f32, tag="sps", name="sps")
                    nc.tensor.matmul(sps[:, :], onesC[:, :], xsq[0][:, fl], start=True, stop=False)
                    nc.tensor.matmul(sps[:, :], onesC[:, :], xsq[1][:, fl], start=False, stop=True)
                    # mu row (SBUF, bf16) + vare = (msq + eps) - mu^2
                    nc.scalar.copy(mu_row[0:1, fl], mps[:, :])
                    mu2 = sml.tile([1, 512], f32, tag="mu2", name="mu2")
                    nc.vector.tensor_tensor(mu2[:, :], mu_row[0:1, fl], mu_row[0:1, fl], ALU.mult)
                    vare = sml.tile([1, 512], f32, tag="vare", name="vare")
                    nc.vector.scalar_tensor_tensor(vare[:, :], sps[:, :], LN_EPS, mu2[:, :],
                                                   ALU.add, ALU.subtract)
                    # rstd = exp(-0.5*log(var+eps)), srt = exp(+0.5*log(var+eps))
                    # (ACT spline tables, <=2 ULP; the custom-DVE fast
                    # reciprocal is broken on this stack)
                    lgv = sml.tile([1, 512], f32, tag="lgv", name="lgv")
                    nc.scalar.activation(lgv[:, :], vare[:, :], ACTF.Ln)
                    nc.scalar.activation(rs_row[0:1, fl], lgv[:, :], ACTF.Exp, scale=-0.5)
                    nc.scalar.activation(rs_bf[0:1, fl], lgv[:, :], ACTF.Exp, scale=-0.5)
                    nc.scalar.activation(srt_row[0:1, fl], lgv[:, :], ACTF.Exp, scale=0.5)
                    # rs broadcast down 128 rows (bf16 matmul) -> SBUF
                    rsb_ps = lnp.tile([128, 512], f32, tag="rsb", name="rsb")
                    nc.tensor.matmul(rsb_ps[:, :], ones_row[:, :], rs_bf[0:1, fl],
                                     start=True, stop=True)
                    nc.vector.tensor_copy(rs_ball[:, fl], rsb_ps[:, :])
                    # rstd row -> column layout (PE transposes, 4 chunks of 128)
                    for t in range(4):
                        j = f * 4 + t
                        nc.tensor.transpose(rsT_ps[:, j:j + 1],
                                            rs_row[0:1, j * 128:(j + 1) * 128], ident[:, :])
                    nc.vector.tensor_copy(rs_cols[:, f * 4:f * 4 + 4],
                                          rsT_ps[:, f * 4:f * 4 + 4])

                    # K projection for this token chunk (both output halves)
                    for co in range(2):
                        cs = slice(co * 128, (co + 1) * 128)
                        ps = mmp.tile([128, 512], f32, tag="proj", name="proj")
                        for ci in range(2):
                            nc.tensor.matmul(ps[:, :], w_sb["k", ci][:, cs],
                                             xb[ci][:, fl], start=(ci == 0), stop=False)
                        nc.tensor.matmul(ps[:, :], wb_sb["k0"][:, cs],
                                         mu_row[0:1, fl], start=False, stop=False)
                        nc.tensor.matmul(ps[:, :], wb_sb["k1"][:, cs],
                                         srt_row[0:1, fl], start=False, stop=True)
                        nc.vector.tensor_tensor(kT[co][:, fl], ps[:, :], rs_ball[:, fl], ALU.mult)

                    # Q projection (only first two chunks = this core's queries)
                    if f < 2:
                        for co in range(2):
                            cs = slice(co * 128, (co + 1) * 128)
                            ps = mmp.tile([128, 512], f32, tag="proj", name="proj")
                            for ci in range(2):
                                nc.tensor.matmul(ps[:, :], w_sb["q", ci][:, cs],
                                                 xb[ci][:, fl], start=(ci == 0), stop=False)
                            nc.tensor.matmul(ps[:, :], wb_sb["q0"][:, cs],
                                             mu_row[0:1, fl], start=False, stop=False)
                            nc.tensor.matmul(ps[:, :], wb_sb["q1"][:, cs],
                                             srt_row[0:1, fl], start=False, stop=True)
                            for h in range(4):
                                rr = slice(h * 32, (h + 1) * 32)
                                nc.vector.tensor_tensor(qp[co][h][rr, fl], ps[rr, :],
                                                        rs_ball[rr, fl], ALU.mult)

                # V projection per 128-token chunk (tokens in partitions). The
                # 33rd "dummy" channel per head has zero weights and rank-1
                # bias = sqrt(var+eps), so after the *rstd evacuation it is
                # exactly the ones column (softmax denominator accumulator).
                for j in range(32):
                    jl = slice(j * 128, (j + 1) * 128)
                    ps = mmp.tile([128, NH * 33], f32, tag="proj", name="vproj")
                    for ci in range(2):
                        nc.tensor.matmul(ps[:, :], xb[ci][:, jl], w_sb["v", ci][:, :],
                                         start=(ci == 0), stop=False)
                    nc.tensor.matmul(ps[:, :], mu_row[0:1, jl],
                                     wb_sb["v0"][:, :], start=False, stop=False)
                    nc.tensor.matmul(ps[:, :], srt_row[0:1, jl],
                                     wb_sb["v1"][:, :], start=False, stop=True)
                    nc.scalar.mul(v_sb[:, j, :, :],
                                  ps[:, :].rearrange("p (h e) -> p h e", h=NH),
                                  rs_cols[:, j:j + 1])

            # ---- attention ----
            with tc.tile_pool(name="sps", bufs=1, space="PSUM") as sp, \
                 tc.tile_pool(name="avp", bufs=1, space="PSUM") as avp, \
                 tc.tile_pool(name="bcp", bufs=2, space="PSUM") as bcp, \
                 tc.tile_pool(name="pp", bufs=2) as ppool, \
                 tc.tile_pool(name="nrm", bufs=2) as nrm:
                for f in range(2):
                    fl = slice(f * 512, (f + 1) * 512)
                    for hg in range(2):
                        av = [avp.tile([128, 512], f32, tag=f"av{pr}", name=f"av{pr}")
                              for pr in range(2)]
                        for j in range(32):
                            jl = slice(j * 128, (j + 1) * 128)
                            ss = [sp.tile([128, 1024], f32, tag=f"s{i}", name=f"s{i}")
                                  for i in range(2)]
                            pt = [ppool.tile([128, 1024], bf16, tag=f"p{i}", name=f"p{i}")
                                  for i in range(2)]
                            for i in range(2):
                                for t2 in range(2):
                                    h = i * 2 + t2
                                    nc.tensor.matmul(ss[i][:, t2 * 512:(t2 + 1) * 512],
                                                     kT[hg][:, jl], qp[hg][h][:, fl],
                                                     start=True, stop=True)
                            # pair 0: true exp on ScalarE; pair 1: Schraudolph on VectorE
                            nc.scalar.activation(pt[0][:, :], ss[0][:, :],
                                                 ACTF.Exp, scale=LN2 / 128.0)
                            nc.vector.tensor_scalar(pt[1][:, :].bitcast(i16), ss[1][:, :],
                                                    B16F, None, ALU.add)
                            for pr in range(2):
                                for t2 in range(2):
                                    h = pr * 2 + t2
                                    nc.tensor.matmul(
                                        av[pr][t2 * 64:t2 * 64 + 33, :],
                                        v_sb[:, j, hg * 4 + h, :],
                                        pt[pr][:, t2 * 512:(t2 + 1) * 512],
                                        start=(j == 0), stop=(j == 31),
                                        tile_position=(0, t2 * 64))
                        # normalization: rcp = 1/denominator (fast approx,
                        # straight off the PSUM ones-row), broadcast, multiply
                        for pr in range(2):
                            for t2 in range(2):
                                lgd = nrm.tile([1, 512], f32, tag="lg", name="lg")
                                nc.scalar.activation(
                                    lgd[:, :], av[pr][t2 * 64 + 32:t2 * 64 + 33, :],
                                    ACTF.Ln)
                                rcpb = nrm.tile([1, 512], bf16, tag="rb", name="rb")
                                nc.scalar.activation(rcpb[:, :], lgd[:, :],
                                                     ACTF.Exp, scale=-1.0)
                                bcq = bcp.tile([32, 512], f32, tag="bcq", name="bcq")
                                nc.tensor.matmul(bcq[:, :], ones_row[:, 0:32],
                                                 rcpb[:, :], start=True, stop=True)
                                bcs = nrm.tile([32, 512], bf16, tag="bcs", name="bcs")
                                nc.vector.tensor_copy(bcs[:, :], bcq[:, :])
                                row0 = (pr * 2 + t2) * 32
                                nc.vector.tensor_tensor(
                                    attnT[hg][row0:row0 + 32, fl],
                                    av[pr][t2 * 64:t2 * 64 + 32, :],
                                    bcs[:, :], ALU.mult)

            # ---- output projection + bias + residual ----
            with tc.tile_pool(name="mm2", bufs=2, space="PSUM") as mm2, \
                 tc.tile_pool(name="ot", bufs=4) as otp:
                for mo in range(2):
                    ms = slice(mo * 128, (mo + 1) * 128)
                    for f in range(2):
                        fl = slice(f * 512, (f + 1) * 512)
                        ps = mm2.tile([128, 512], f32, tag="o", name="o")
                        for ci in range(2):
                            nc.tensor.matmul(ps[:, :], w_sb["p", ci][:, ms],
                                             attnT[ci][:, fl], start=(ci == 0), stop=(ci == 1))
                        ot = otp.tile([128, 512], f32, tag="ot", name="ot")
                        nc.vector.scalar_tensor_tensor(ot[:, :], ps[:, :], bp_sb[mo][:, :],
                                                       xf[mo][:, fl], ALU.add, ALU.add)
                        nc.sync.dma_start(out=od[ms, fl], in_=ot[:, :])

    nc.finalize()
    return nc


def kernel(x, ln_gamma, ln_beta, w_qkv, w_proj, b_proj):
    import ml_dtypes
    from concourse.bass_utils import run_bass_kernel_spmd

    if "nc" not in _CACHE:
        _CACHE["nc"] = _build()
    nc = _CACHE["nc"]

    bf = ml_dtypes.bfloat16
    x = np.asarray(x, np.float32)
    w_qkv = np.asarray(w_qkv, np.float32)
    gam = np.asarray(ln_gamma, np.float32)
    bet = np.asarray(ln_beta, np.float32)
    wq_, wk_, wv_ = w_qkv[0:C], w_qkv[C:2 * C], w_qkv[2 * C:3 * C]

    def prep(wmat, scale):
        wg = (scale * wmat * gam[None, :]).astype(bf)           # [o, c] gamma folded
        wT = np.ascontiguousarray(wg.T)                         # lhsT layout [in, out]
        sw = wg.astype(np.float32).sum(1)                       # rowsum of device weights
        bias = scale * (wmat @ bet)
        return (wT, np.ascontiguousarray(-sw[None, :].astype(bf)),
                np.ascontiguousarray(bias[None, :].astype(bf)))

    wqT, wbq0_h, wbq1_h = prep(wq_, A_SCALE)
    wkT, wbk0_h, wbk1_h = prep(wk_, 1.0)
    # V extended with a zero-weight dummy channel per head whose rank-1 bias
    # is 1 against the srt row (becomes the softmax-denominator ones column).
    wvg = (wv_ * gam[None, :]).astype(bf)
    wv_ext = np.zeros((NH * 33, C), bf)
    wbv0_h = np.zeros((1, NH * 33), np.float32)
    wbv1_h = np.zeros((1, NH * 33), np.float32)
    for h in range(NH):
        wv_ext[h * 33:h * 33 + 32] = wvg[h * 32:(h + 1) * 32]
        wbv0_h[0, h * 33:h * 33 + 32] = -wvg[h * 32:(h + 1) * 32].astype(np.float32).sum(1)
        wbv1_h[0, h * 33:h * 33 + 32] = (wv_ @ bet)[h * 32:(h + 1) * 32]
        wbv1_h[0, h * 33 + 32] = 1.0
    wvT = np.ascontiguousarray(wv_ext.T)
    wbv0_h = wbv0_h.astype(bf)
    wbv1_h = wbv1_h.astype(bf)
    wpT = np.ascontiguousarray(np.asarray(w_proj, np.float32).T.astype(bf))
    bp = np.asarray(b_proj, np.float32).reshape(C, 1)

    xfull = x.reshape(B, C, N)
    in_maps = []
    for core in range(8):
        b, qc = core // 4, core % 4
        xr = np.roll(xfull[b], -qc * NQ, axis=1)
        in_maps.append({
            "xb": np.ascontiguousarray(xr.astype(bf)),
            "xf": np.ascontiguousarray(xr[:, :NQ]),
            "wqT": wqT, "wkT": wkT, "wvT": wvT, "wpT": wpT,
            "wbq0": wbq0_h, "wbq1": wbq1_h, "wbk0": wbk0_h, "wbk1": wbk1_h,
            "wbv0": wbv0_h, "wbv1": wbv1_h, "bp": bp,
        })

    res = run_bass_kernel_spmd(nc, in_maps, core_ids=list(range(8)),
                               trace=_PROFILE)
    if _PROFILE:
        _CACHE["exec_time_ns"] = res.exec_time_ns
    out = np.empty((B, C, N), np.float32)
    for core in range(8):
        b, qc = core // 4, core % 4
        out[b][:, qc * NQ:(qc + 1) * NQ] = res.results[core]["out"]
    return out.reshape(B, C, H, W)


# revision 34
# speedup vs baseline: 1.4268x; 1.2088x over previous
"""Trainium2 Bass kernel for nn_Attention (B=2, C=256, H=W=64, 8 heads).

Sharding: 8 cores = 2 batches x 4 query-chunks (1024 queries each), no
collectives. Each core gets its batch's full x (bf16) with token columns
rolled so its own query chunk sits at columns 0:1024 (attention is
permutation-invariant over keys); it computes LN + projections + attention
for its queries and writes a [256, 1024] slice of the output.

Key structure (v2 — rebuilt for PE throughput):
- LN is folded into the projections: gamma is pre-multiplied into the
  weights on the host; the per-token mean/rstd enter as a rank-1 fixup
  matmul accumulated into each projection's PSUM (lhsT = [-rowsum(W); W@beta],
  rhs = [mu; sqrt(var+eps)]) followed by a *rstd multiply at PSUM
  evacuation. No normalized-x tensor is ever materialized.
- S^T matmuls are full-array (unmasked): the stationary is the whole
  4-head K chunk [128x128]; per-head Q lives in zero-padded [128, NQ]
  tiles so each 512-query matmul contracts over all 128 channel rows but
  only the head's 32 rows are nonzero. This keeps the PE HAM clock-gate
  warm (masked tile_position matmuls don't count as PE activity) and
  shares one LDWEIGHTS across the 4 S matmuls of a key chunk.
- exp splits per key-chunk between ScalarE (true exp on head-pair 0) and
  VectorE (Schraudolph bit-trick on head-pair 1): Wq is pre-scaled so
  PSUM = 128*log2(e)*logit; adding a magic bias and converting f32->int16
  yields the bf16 bit pattern of 2^y (~3% max rel err, harmless next to
  the residual). Both pairs process [128,1024] two-bank PSUM tiles.
- P^T V via stationary-V matmuls with an appended ones column gives the
  softmax denominators for free; normalization uses the fast custom-DVE
  reciprocal (~5x faster than the 8-pass DIV) broadcast through a tiny
  f32r matmul.
"""

import numpy as np

B, C, H, W = 2, 256, 64, 64
N = H * W            # 4096 tokens
NH, HD = 8, 32       # heads, head_dim
NQ = N // 4          # queries per core
LN_EPS = 1e-5
LOG2E = 1.4426950408889634
LN2 = 0.6931471805599453
ATTN_SCALE = HD ** -0.5
A_SCALE = 128.0 * LOG2E * ATTN_SCALE   # folded into Wq on host
B16F = 16256.0 - 5.6                   # Schraudolph bias (calibrated)
KRSQ = 24375.25                        # bf16 bit-trick rsqrt bias (<=3.7% rel)
KSQ = 8123.0                           # bf16 bit-trick sqrt bias (<=3.9% rel)
KRCP = 32498.75                        # bf16 bit-trick recip bias (<=5.3% rel)

_PROFILE = False
_CACHE = {}


def _build():
    from concourse import bacc
    from concourse import mybir
    import concourse.tile as tile

    f32 = mybir.dt.float32
    f32r = mybir.dt.float32r
    bf16 = mybir.dt.bfloat16
    i16 = mybir.dt.int16
    ALU = mybir.AluOpType
    ACTF = mybir.ActivationFunctionType

    nc = bacc.Bacc("TRN2", target_bir_lowering=False)
    xbd = nc.dram_tensor("xb", [C, N], bf16, kind="ExternalInput")
    xfd = nc.dram_tensor("xf", [C, NQ], f32, kind="ExternalInput")
    wq = nc.dram_tensor("wqT", [C, C], bf16, kind="ExternalInput")  # gamma+A_SCALE folded
    wk = nc.dram_tensor("wkT", [C, C], bf16, kind="ExternalInput")
    wv = nc.dram_tensor("wvT", [C, NH * 33], bf16, kind="ExternalInput")
    wp = nc.dram_tensor("wpT", [C, C], bf16, kind="ExternalInput")
    # rank-1 LN fixup rows: *0 = -rowsum(W'), *1 = W@beta (each own tensor so
    # every engine/matmul access starts at partition 0)
    wbq0 = nc.dram_tensor("wbq0", [1, C], bf16, kind="ExternalInput")
    wbq1 = nc.dram_tensor("wbq1", [1, C], bf16, kind="ExternalInput")
    wbk0 = nc.dram_tensor("wbk0", [1, C], bf16, kind="ExternalInput")
    wbk1 = nc.dram_tensor("wbk1", [1, C], bf16, kind="ExternalInput")
    wbv0 = nc.dram_tensor("wbv0", [1, NH * 33], bf16, kind="ExternalInput")
    wbv1 = nc.dram_tensor("wbv1", [1, NH * 33], bf16, kind="ExternalInput")
    bpd = nc.dram_tensor("bp", [C, 1], f32, kind="ExternalInput")
    od = nc.dram_tensor("out", [C, NQ], f32, kind="ExternalOutput")

    with tile.TileContext(nc) as tc:
        with tc.tile_pool(name="big", bufs=1) as big, \
             tc.tile_pool(name="sml", bufs=2) as sml:

            # ---- load inputs ----
            xb = [big.tile([128, N], bf16, tag=f"xb{c}", name=f"xb{c}") for c in range(2)]
            for c in range(2):
                nc.sync.dma_start(out=xb[c][:, :], in_=xbd[c * 128:(c + 1) * 128, :])
            xf = [big.tile([128, NQ], f32, tag=f"xf{c}", name=f"xf{c}") for c in range(2)]
            for c in range(2):
                nc.sync.dma_start(out=xf[c][:, :], in_=xfd[c * 128:(c + 1) * 128, :])
            w_sb = {}
            for name, t, nout in (("q", wq, C), ("k", wk, C),
                                  ("v", wv, NH * 33), ("p", wp, C)):
                for c in range(2):
                    s = big.tile([128, nout], bf16, tag=f"w{name}{c}", name=f"w{name}{c}")
                    nc.sync.dma_start(out=s[:, :], in_=t[c * 128:(c + 1) * 128, :])
                    w_sb[name, c] = s
            wb_sb = {}
            for name, t, nout in (("q0", wbq0, C), ("q1", wbq1, C),
                                  ("k0", wbk0, C), ("k1", wbk1, C),
                                  ("v0", wbv0, NH * 33), ("v1", wbv1, NH * 33)):
                s = big.tile([1, nout], bf16, tag=f"wb{name}", name=f"wb{name}")
                nc.sync.dma_start(out=s[:, :], in_=t[:, :])
                wb_sb[name] = s
            bp_sb = [big.tile([128, 1], f32, tag=f"bp{c}", name=f"bp{c}") for c in range(2)]
            for c in range(2):
                nc.sync.dma_start(out=bp_sb[c][:, :], in_=bpd[c * 128:(c + 1) * 128, :])

            onesC = big.tile([128, 1], bf16, tag="onesC", name="onesC")
            nc.vector.memset(onesC[:, :], 1.0 / C)
            ones_row = big.tile([1, 128], bf16, tag="onesr", name="onesr")
            nc.vector.memset(ones_row[:, :], 1.0)
            ident = big.tile([1, 1], f32, tag="ident", name="ident")
            nc.vector.memset(ident[:, :], 1.0)

            rs_cols = big.tile([128, 32], f32, tag="rscols", name="rscols")  # rstd, col layout

            kT = [big.tile([128, N], bf16, tag=f"kT{c}", name=f"kT{c}") for c in range(2)]
            qp = [[big.tile([128, NQ], bf16, tag=f"qp{hg}{h}", name=f"qp{hg}{h}")
                   for h in range(4)] for hg in range(2)]
            for hg in range(2):
                for h in range(4):
                    nc.gpsimd.memset(qp[hg][h][:, :], 0.0)
            v_sb = big.tile([128, 32, NH, 33], bf16, tag="v", name="v")
            attnT = [big.tile([128, NQ], bf16, tag=f"at{c}", name=f"at{c}") for c in range(2)]

            # ---- LN stats + projections ----
            with tc.tile_pool(name="lnsb", bufs=1) as lnsb, \
                 tc.tile_pool(name="lnp", bufs=1, space="PSUM") as lnp, \
                 tc.tile_pool(name="mm", bufs=2, space="PSUM") as mmp:
                mu_row = lnsb.tile([1, N], bf16, tag="murow", name="murow")
                srt_row = lnsb.tile([1, N], bf16, tag="srtrow", name="srtrow")
                rs_row = lnsb.tile([1, N], f32, tag="rsrow", name="rsrow")
                rs_bf = lnsb.tile([1, N], bf16, tag="rsbf", name="rsbf")
                rs_ball = lnsb.tile([128, N], f32, tag="rsball", name="rsball")
                xsq = [lnsb.tile([128, N], bf16, tag=f"xsq{c}", name=f"xsq{c}") for c in range(2)]
                nc.vector.tensor_tensor(xsq[0][:, :], xb[0][:, :], xb[0][:, :], ALU.mult)
                nc.vector.tensor_tensor(xsq[1][:, :], xb[1][:, :], xb[1][:, :], ALU.mult)
                rsT_ps = lnp.tile([128, 32], f32, tag="rsT", name="rsT")
                for f in range(8):
                    fl = slice(f * 512, (f + 1) * 512)
                    mps = lnp.tile([1, 512], f32, tag="mps", name="mps")
                    nc.tensor.matmul(mps[:, :], onesC[:, :], xb[0][:, fl], start=True, stop=False)
                    nc.tensor.matmul(mps[:, :], onesC[:, :], xb[1][:, fl], start=False, stop=True)
                    sps = lnp.tile([1, 512], f32, tag="sps", name="sps")
                    nc.tensor.matmul(sps[:, :], onesC[:, :], xsq[0][:, fl], start=True, stop=False)
                    nc.tensor.matmul(sps[:, :], onesC[:, :], xsq[1][:, fl], start=False, stop=True)
                    # mu row (SBUF, bf16) + vare = (msq + eps) - mu^2
                    nc.scalar.copy(mu_row[0:1, fl], mps[:, :])
                    mu2 = sml.tile([1, 512], f32, tag="mu2", name="mu2")
                    nc.vector.tensor_tensor(mu2[:, :], mu_row[0:1, fl], mu_row[0:1, fl], ALU.mult)
                    vare = sml.tile([1, 512], f32, tag="vare", name="vare")
                    nc.vector.scalar_tensor_tensor(vare[:, :], sps[:, :], LN_EPS, mu2[:, :],
                                                   ALU.add, ALU.subtract)
                    # rstd / sqrt via bf16 exponent bit tricks on VectorE
                    # (keeps ScalarE's ACT table pinned to the Exp set; the
                    # custom-DVE fast reciprocal is broken on this stack)
                    vb = sml.tile([1, 512], bf16, tag="vb", name="vb")
                    nc.vector.tensor_copy(vb[:, :], vare[:, :])
                    nc.vector.tensor_scalar(rs_bf[0:1, fl].bitcast(i16),
                                            vb[:, :].bitcast(i16),
                                            -0.5, KRSQ, ALU.mult, ALU.add)
                    nc.vector.tensor_scalar(srt_row[0:1, fl].bitcast(i16),
                                            vb[:, :].bitcast(i16),
                                            0.5, KSQ, ALU.mult, ALU.add)
                    nc.scalar.copy(rs_row[0:1, fl], rs_bf[0:1, fl])
                    # rs broadcast down 128 rows (bf16 matmul) -> SBUF
                    rsb_ps = lnp.tile([128, 512], f32, tag="rsb", name="rsb")
                    nc.tensor.matmul(rsb_ps[:, :], ones_row[:, :], rs_bf[0:1, fl],
                                     start=True, stop=True)
                    nc.vector.tensor_copy(rs_ball[:, fl], rsb_ps[:, :])
                    # rstd row -> column layout (PE transposes, 4 chunks of 128)
                    for t in range(4):
                        j = f * 4 + t
                        nc.tensor.transpose(rsT_ps[:, j:j + 1],
                                            rs_row[0:1, j * 128:(j + 1) * 128], ident[:, :])
                    nc.vector.tensor_copy(rs_cols[:, f * 4:f * 4 + 4],
                                          rsT_ps[:, f * 4:f * 4 + 4])

                    # K projection for this token chunk (both output halves)
                    for co in range(2):
                        cs = slice(co * 128, (co + 1) * 128)
                        ps = mmp.tile([128, 512], f32, tag="proj", name="proj")
                        for ci in range(2):
                            nc.tensor.matmul(ps[:, :], w_sb["k", ci][:, cs],
                                             xb[ci][:, fl], start=(ci == 0), stop=False)
                        nc.tensor.matmul(ps[:, :], wb_sb["k0"][:, cs],
                                         mu_row[0:1, fl], start=False, stop=False)
                        nc.tensor.matmul(ps[:, :], wb_sb["k1"][:, cs],
                                         srt_row[0:1, fl], start=False, stop=True)
                        nc.vector.tensor_tensor(kT[co][:, fl], ps[:, :], rs_ball[:, fl], ALU.mult)

                    # Q projection (only first two chunks = this core's queries)
                    if f < 2:
                        for co in range(2):
                            cs = slice(co * 128, (co + 1) * 128)
                            ps = mmp.tile([128, 512], f32, tag="proj", name="proj")
                            for ci in range(2):
                                nc.tensor.matmul(ps[:, :], w_sb["q", ci][:, cs],
                                                 xb[ci][:, fl], start=(ci == 0), stop=False)
                            nc.tensor.matmul(ps[:, :], wb_sb["q0"][:, cs],
                                             mu_row[0:1, fl], start=False, stop=False)
                            nc.tensor.matmul(ps[:, :], wb_sb["q1"][:, cs],
                                             srt_row[0:1, fl], start=False, stop=True)
                            for h in range(4):
                                rr = slice(h * 32, (h + 1) * 32)
                                nc.vector.tensor_tensor(qp[co][h][rr, fl], ps[rr, :],
                                                        rs_ball[rr, fl], ALU.mult)

                # V projection per 128-token chunk (tokens in partitions). The
                # 33rd "dummy" channel per head has zero weights and rank-1
                # bias = sqrt(var+eps), so after the *rstd evacuation it is
                # exactly the ones column (softmax denominator accumulator).
                for j in range(32):
                    jl = slice(j * 128, (j + 1) * 128)
                    ps = mmp.tile([128, NH * 33], f32, tag="proj", name="vproj")
                    for ci in range(2):
                        nc.tensor.matmul(ps[:, :], xb[ci][:, jl], w_sb["v", ci][:, :],
                                         start=(ci == 0), stop=False)
                    nc.tensor.matmul(ps[:, :], mu_row[0:1, jl],
                                     wb_sb["v0"][:, :], start=False, stop=False)
                    nc.tensor.matmul(ps[:, :], srt_row[0:1, jl],
                                     wb_sb["v1"][:, :], start=False, stop=True)
                    nc.scalar.mul(v_sb[:, j, :, :],
                                  ps[:, :].rearrange("p (h e) -> p h e", h=NH),
                                  rs_cols[:, j:j + 1])

            # ---- attention ----
            with tc.tile_pool(name="sps", bufs=1, space="PSUM") as sp, \
                 tc.tile_pool(name="avp", bufs=1, space="PSUM") as avp, \
                 tc.tile_pool(name="bcp", bufs=2, space="PSUM") as bcp, \
                 tc.tile_pool(name="pp", bufs=2) as ppool, \
                 tc.tile_pool(name="nrm", bufs=2) as nrm:
                for f in range(2):
                    fl = slice(f * 512, (f + 1) * 512)
                    for hg in range(2):
                        av = [avp.tile([128, 512], f32, tag=f"av{pr}", name=f"av{pr}")
                              for pr in range(2)]
                        # S(j) interleaves with AV(j-1) so each S matmul's
                        # 128-col LDWEIGHTS hides under the preceding AV matmul
                        prev_pt = None
                        for j in range(32):
                            jl = slice(j * 128, (j + 1) * 128)
                            ss = [sp.tile([128, 1024], f32, tag=f"s{i}", name=f"s{i}")
                                  for i in range(2)]
                            pt = [ppool.tile([128, 1024], bf16, tag=f"p{i}", name=f"p{i}")
                                  for i in range(2)]
                            for i in range(2):
                                for t2 in range(2):
                                    h = i * 2 + t2
                                    nc.tensor.matmul(ss[i][:, t2 * 512:(t2 + 1) * 512],
                                                     kT[hg][:, jl], qp[hg][h][:, fl],
                                                     start=True, stop=True)
                                    if prev_pt is not None:
                                        nc.tensor.matmul(
                                            av[i][t2 * 64:t2 * 64 + 33, :],
                                            v_sb[:, j - 1, hg * 4 + h, :],
                                            prev_pt[i][:, t2 * 512:(t2 + 1) * 512],
                                            start=(j == 1), stop=False,
                                            tile_position=(0, t2 * 64))
                            # pair 0: true exp on ScalarE; pair 1: Schraudolph on VectorE
                            nc.scalar.activation(pt[0][:, :], ss[0][:, :],
                                                 ACTF.Exp, scale=LN2 / 128.0)
                            nc.vector.tensor_scalar(pt[1][:, :].bitcast(i16), ss[1][:, :],
                                                    B16F, None, ALU.add)
                            prev_pt = pt
                        for pr in range(2):
                            for t2 in range(2):
                                h = pr * 2 + t2
                                nc.tensor.matmul(
                                    av[pr][t2 * 64:t2 * 64 + 33, :],
                                    v_sb[:, 31, hg * 4 + h, :],
                                    prev_pt[pr][:, t2 * 512:(t2 + 1) * 512],
                                    start=False, stop=True,
                                    tile_position=(0, t2 * 64))
                        # normalization: bit-trick reciprocal of the PSUM
                        # ones-row denominator, broadcast, multiply
                        for pr in range(2):
                            for t2 in range(2):
                                db = nrm.tile([1, 512], bf16, tag="db", name="db")
                                nc.vector.tensor_copy(
                                    db[:, :], av[pr][t2 * 64 + 32:t2 * 64 + 33, :])
                                rcpb = nrm.tile([1, 512], bf16, tag="rb", name="rb")
                                nc.vector.tensor_scalar(rcpb[:, :].bitcast(i16),
                                                        db[:, :].bitcast(i16),
                                                        -1.0, KRCP, ALU.mult, ALU.add)
                                bcq = bcp.tile([32, 512], f32, tag="bcq", name="bcq")
                                nc.tensor.matmul(bcq[:, :], ones_row[:, 0:32],
                                                 rcpb[:, :], start=True, stop=True)
                                bcs = nrm.tile([32, 512], bf16, tag="bcs", name="bcs")
                                nc.vector.tensor_copy(bcs[:, :], bcq[:, :])
                                row0 = (pr * 2 + t2) * 32
                                nc.vector.tensor_tensor(
                                    attnT[hg][row0:row0 + 32, fl],
                                    av[pr][t2 * 64:t2 * 64 + 32, :],
                                    bcs[:, :], ALU.mult)

            # ---- output projection + bias + residual ----
            with tc.tile_pool(name="mm2", bufs=2, space="PSUM") as mm2, \
                 tc.tile_pool(name="ot", bufs=4) as otp:
                for mo in range(2):
                    ms = slice(mo * 128, (mo + 1) * 128)
                    for f in range(2):
                        fl = slice(f * 512, (f + 1) * 512)
                        ps = mm2.tile([128, 512], f32, tag="o", name="o")
                        for ci in range(2):
                            nc.tensor.matmul(ps[:, :], w_sb["p", ci][:, ms],
                                             attnT[ci][:, fl], start=(ci == 0), stop=(ci == 1))
                        ot = otp.tile([128, 512], f32, tag="ot", name="ot")
                        nc.vector.scalar_tensor_tensor(ot[:, :], ps[:, :], bp_sb[mo][:, :],
                                                       xf[mo][:, fl], ALU.add, ALU.add)
                        nc.sync.dma_start(out=od[ms, fl], in_=ot[:, :])

    nc.finalize()
    return nc


def kernel(x, ln_gamma, ln_beta, w_qkv, w_proj, b_proj):
    import ml_dtypes
    from concourse.bass_utils import run_bass_kernel_spmd

    if "nc" not in _CACHE:
        _CACHE["nc"] = _build()
    nc = _CACHE["nc"]

    bf = ml_dtypes.bfloat16
    x = np.asarray(x, np.float32)
    w_qkv = np.asarray(w_qkv, np.float32)
    gam = np.asarray(ln_gamma, np.float32)
    bet = np.asarray(ln_beta, np.float32)
    wq_, wk_, wv_ = w_qkv[0:C], w_qkv[C:2 * C], w_qkv[2 * C:3 * C]

    def prep(wmat, scale):
        wg = (scale * wmat * gam[None, :]).astype(bf)           # [o, c] gamma folded
        wT = np.ascontiguousarray(wg.T)                         # lhsT layout [in, out]
        sw = wg.astype(np.float32).sum(1)                       # rowsum of device weights
        bias = scale * (wmat @ bet)
        return (wT, np.ascontiguousarray(-sw[None, :].astype(bf)),
                np.ascontiguousarray(bias[None, :].astype(bf)))

    wqT, wbq0_h, wbq1_h = prep(wq_, A_SCALE)
    wkT, wbk0_h, wbk1_h = prep(wk_, 1.0)
    # V extended with a zero-weight dummy channel per head whose rank-1 bias
    # is 1 against the srt row (becomes the softmax-denominator ones column).
    wvg = (wv_ * gam[None, :]).astype(bf)
    wv_ext = np.zeros((NH * 33, C), bf)
    wbv0_h = np.zeros((1, NH * 33), np.float32)
    wbv1_h = np.zeros((1, NH * 33), np.float32)
    for h in range(NH):
        wv_ext[h * 33:h * 33 + 32] = wvg[h * 32:(h + 1) * 32]
        wbv0_h[0, h * 33:h * 33 + 32] = -wvg[h * 32:(h + 1) * 32].astype(np.float32).sum(1)
        wbv1_h[0, h * 33:h * 33 + 32] = (wv_ @ bet)[h * 32:(h + 1) * 32]
        wbv1_h[0, h * 33 + 32] = 1.0
    wvT = np.ascontiguousarray(wv_ext.T)
    wbv0_h = wbv0_h.astype(bf)
    wbv1_h = wbv1_h.astype(bf)
    wpT = np.ascontiguousarray(np.asarray(w_proj, np.float32).T.astype(bf))
    bp = np.asarray(b_proj, np.float32).reshape(C, 1)

    xfull = x.reshape(B, C, N)
    in_maps = []
    for core in range(8):
        b, qc = core // 4, core % 4
        xr = np.roll(xfull[b], -qc * NQ, axis=1)
        in_maps.append({
            "xb": np.ascontiguousarray(xr.astype(bf)),
            "xf": np.ascontiguousarray(xr[:, :NQ]),
            "wqT": wqT, "wkT": wkT, "wvT": wvT, "wpT": wpT,
            "wbq0": wbq0_h, "wbq1": wbq1_h, "wbk0": wbk0_h, "wbk1": wbk1_h,
            "wbv0": wbv0_h, "wbv1": wbv1_h, "bp": bp,
        })

    res = run_bass_kernel_spmd(nc, in_maps, core_ids=list(range(8)),
                               trace=_PROFILE)
    if _PROFILE:
        _CACHE["exec_time_ns"] = res.exec_time_ns
    out = np.empty((B, C, N), np.float32)
    for core in range(8):
        b, qc = core // 4, core % 4
        out[b][:, qc * NQ:(qc + 1) * NQ] = res.results[core]["out"]
    return out.reshape(B, C, H, W)


# revision 36
# speedup vs baseline: 1.4944x; 1.0474x over previous
"""Trainium2 Bass kernel for nn_Attention (B=2, C=256, H=W=64, 8 heads).

Sharding: 8 cores = 2 batches x 4 query-chunks (1024 queries each), no
collectives. Each core gets its batch's full x (bf16) with token columns
rolled so its own query chunk sits at columns 0:1024 (attention is
permutation-invariant over keys); it computes LN + projections + attention
for its queries and writes a [256, 1024] slice of the output.

Key structure (v2 — rebuilt for PE throughput):
- LN is folded into the projections: gamma is pre-multiplied into the
  weights on the host; the per-token mean/rstd enter as a rank-1 fixup
  matmul accumulated into each projection's PSUM (lhsT = [-rowsum(W); W@beta],
  rhs = [mu; sqrt(var+eps)]) followed by a *rstd multiply at PSUM
  evacuation. No normalized-x tensor is ever materialized.
- S^T matmuls are full-array (unmasked): the stationary is the whole
  4-head K chunk [128x128]; per-head Q lives in zero-padded [128, NQ]
  tiles so each 512-query matmul contracts over all 128 channel rows but
  only the head's 32 rows are nonzero. This keeps the PE HAM clock-gate
  warm (masked tile_position matmuls don't count as PE activity) and
  shares one LDWEIGHTS across the 4 S matmuls of a key chunk.
- exp splits per key-chunk between ScalarE (true exp on head-pair 0) and
  VectorE (Schraudolph bit-trick on head-pair 1): Wq is pre-scaled so
  PSUM = 128*log2(e)*logit; adding a magic bias and converting f32->int16
  yields the bf16 bit pattern of 2^y (~3% max rel err, harmless next to
  the residual). Both pairs process [128,1024] two-bank PSUM tiles.
- P^T V via stationary-V matmuls with an appended ones column gives the
  softmax denominators for free; normalization uses the fast custom-DVE
  reciprocal (~5x faster than the 8-pass DIV) broadcast through a tiny
  f32r matmul.
"""

import numpy as np

B, C, H, W = 2, 256, 64, 64
N = H * W            # 4096 tokens
NH, HD = 8, 32       # heads, head_dim
NQ = N // 4          # queries per core
LN_EPS = 1e-5
LOG2E = 1.4426950408889634
LN2 = 0.6931471805599453
ATTN_SCALE = HD ** -0.5
A_SCALE = 128.0 * LOG2E * ATTN_SCALE   # folded into Wq on host
B16F = 16256.0 - 5.6                   # Schraudolph bias (calibrated)
KRSQ = 24375.25                        # bf16 bit-trick rsqrt bias (<=3.7% rel)
KSQ = 8123.0                           # bf16 bit-trick sqrt bias (<=3.9% rel)
KRCP = 32498.75                        # bf16 bit-trick recip bias (<=5.3% rel)

_PROFILE = False
_CACHE = {}


def _build():
    from concourse import bacc
    from concourse import mybir
    import concourse.tile as tile

    f32 = mybir.dt.float32
    f32r = mybir.dt.float32r
    bf16 = mybir.dt.bfloat16
    i16 = mybir.dt.int16
    ALU = mybir.AluOpType
    ACTF = mybir.ActivationFunctionType

    nc = bacc.Bacc("TRN2", target_bir_lowering=False)
    xbd = nc.dram_tensor("xb", [C, N], bf16, kind="ExternalInput")
    xfd = nc.dram_tensor("xf", [C, NQ], f32, kind="ExternalInput")
    wq = nc.dram_tensor("wqT", [C, C], bf16, kind="ExternalInput")  # gamma+A_SCALE folded
    wk = nc.dram_tensor("wkT", [C, C], bf16, kind="ExternalInput")
    wv = nc.dram_tensor("wvT", [C, NH * 33], bf16, kind="ExternalInput")
    wp = nc.dram_tensor("wpT", [C, C], bf16, kind="ExternalInput")
    # rank-1 LN fixup rows: *0 = -rowsum(W'), *1 = W@beta (each own tensor so
    # every engine/matmul access starts at partition 0)
    wbq0 = nc.dram_tensor("wbq0", [1, C], bf16, kind="ExternalInput")
    wbq1 = nc.dram_tensor("wbq1", [1, C], bf16, kind="ExternalInput")
    wbk0 = nc.dram_tensor("wbk0", [1, C], bf16, kind="ExternalInput")
    wbk1 = nc.dram_tensor("wbk1", [1, C], bf16, kind="ExternalInput")
    wbv0 = nc.dram_tensor("wbv0", [1, NH * 33], bf16, kind="ExternalInput")
    wbv1 = nc.dram_tensor("wbv1", [1, NH * 33], bf16, kind="ExternalInput")
    bpd = nc.dram_tensor("bp", [C, 1], f32, kind="ExternalInput")
    od = nc.dram_tensor("out", [C, NQ], f32, kind="ExternalOutput")

    with tile.TileContext(nc) as tc:
        with tc.tile_pool(name="big", bufs=1) as big, \
             tc.tile_pool(name="sml", bufs=2) as sml:

            # ---- load inputs ----
            xb = [big.tile([128, N], bf16, tag=f"xb{c}", name=f"xb{c}") for c in range(2)]
            for q4 in range(4):
                qs = slice(q4 * 1024, (q4 + 1) * 1024)
                for c in range(2):
                    nc.sync.dma_start(out=xb[c][:, qs], in_=xbd[c * 128:(c + 1) * 128, qs])
            xf = [big.tile([128, NQ], f32, tag=f"xf{c}", name=f"xf{c}") for c in range(2)]
            for c in range(2):
                nc.sync.dma_start(out=xf[c][:, :], in_=xfd[c * 128:(c + 1) * 128, :])
            w_sb = {}
            for name, t, nout in (("q", wq, C), ("k", wk, C),
                                  ("v", wv, NH * 33), ("p", wp, C)):
                for c in range(2):
                    s = big.tile([128, nout], bf16, tag=f"w{name}{c}", name=f"w{name}{c}")
                    nc.sync.dma_start(out=s[:, :], in_=t[c * 128:(c + 1) * 128, :])
                    w_sb[name, c] = s
            wb_sb = {}
            for name, t, nout in (("q0", wbq0, C), ("q1", wbq1, C),
                                  ("k0", wbk0, C), ("k1", wbk1, C),
                                  ("v0", wbv0, NH * 33), ("v1", wbv1, NH * 33)):
                s = big.tile([1, nout], bf16, tag=f"wb{name}", name=f"wb{name}")
                nc.sync.dma_start(out=s[:, :], in_=t[:, :])
                wb_sb[name] = s
            bp_sb = [big.tile([128, 1], f32, tag=f"bp{c}", name=f"bp{c}") for c in range(2)]
            for c in range(2):
                nc.sync.dma_start(out=bp_sb[c][:, :], in_=bpd[c * 128:(c + 1) * 128, :])

            onesC = big.tile([128, 1], bf16, tag="onesC", name="onesC")
            nc.vector.memset(onesC[:, :], 1.0 / C)
            ones_row = big.tile([1, 128], bf16, tag="onesr", name="onesr")
            nc.vector.memset(ones_row[:, :], 1.0)
            ident = big.tile([1, 1], f32, tag="ident", name="ident")
            nc.vector.memset(ident[:, :], 1.0)

            rs_cols = big.tile([128, 32], f32, tag="rscols", name="rscols")  # rstd, col layout

            kT = [big.tile([128, N], bf16, tag=f"kT{c}", name=f"kT{c}") for c in range(2)]
            qp = [[big.tile([128, NQ], bf16, tag=f"qp{hg}{h}", name=f"qp{hg}{h}")
                   for h in range(4)] for hg in range(2)]
            for hg in range(2):
                for h in range(4):
                    nc.gpsimd.memset(qp[hg][h][:, :], 0.0)
            v_sb = big.tile([128, 32, NH, 33], bf16, tag="v", name="v")
            attnT = [big.tile([128, NQ], bf16, tag=f"at{c}", name=f"at{c}") for c in range(2)]

            # ---- LN stats + projections ----
            with tc.tile_pool(name="lnsb", bufs=1) as lnsb, \
                 tc.tile_pool(name="lnp", bufs=1, space="PSUM") as lnp, \
                 tc.tile_pool(name="mm", bufs=2, space="PSUM") as mmp:
                mu_row = lnsb.tile([1, N], bf16, tag="murow", name="murow")
                srt_row = lnsb.tile([1, N], bf16, tag="srtrow", name="srtrow")
                rs_row = lnsb.tile([1, N], f32, tag="rsrow", name="rsrow")
                rs_bf = lnsb.tile([1, N], bf16, tag="rsbf", name="rsbf")
                rs_ball = lnsb.tile([128, N], f32, tag="rsball", name="rsball")
                xsq = [lnsb.tile([128, N], bf16, tag=f"xsq{c}", name=f"xsq{c}") for c in range(2)]
                nc.vector.tensor_tensor(xsq[0][:, :], xb[0][:, :], xb[0][:, :], ALU.mult)
                nc.vector.tensor_tensor(xsq[1][:, :], xb[1][:, :], xb[1][:, :], ALU.mult)
                rsT_ps = lnp.tile([128, 32], f32, tag="rsT", name="rsT")
                for f in range(8):
                    fl = slice(f * 512, (f + 1) * 512)
                    mps = lnp.tile([1, 512], f32, tag="mps", name="mps")
                    nc.tensor.matmul(mps[:, :], onesC[:, :], xb[0][:, fl], start=True, stop=False)
                    nc.tensor.matmul(mps[:, :], onesC[:, :], xb[1][:, fl], start=False, stop=True)
                    sps = lnp.tile([1, 512], f32, tag="sps", name="sps")
                    nc.tensor.matmul(sps[:, :], onesC[:, :], xsq[0][:, fl], start=True, stop=False)
                    nc.tensor.matmul(sps[:, :], onesC[:, :], xsq[1][:, fl], start=False, stop=True)
                    # mu row (SBUF, bf16) + vare = (msq + eps) - mu^2
                    nc.scalar.copy(mu_row[0:1, fl], mps[:, :])
                    mu2 = sml.tile([1, 512], f32, tag="mu2", name="mu2")
                    nc.vector.tensor_tensor(mu2[:, :], mu_row[0:1, fl], mu_row[0:1, fl], ALU.mult)
                    vare = sml.tile([1, 512], f32, tag="vare", name="vare")
                    nc.vector.scalar_tensor_tensor(vare[:, :], sps[:, :], LN_EPS, mu2[:, :],
                                                   ALU.add, ALU.subtract)
                    # rstd / sqrt via bf16 exponent bit tricks on VectorE
                    # (keeps ScalarE's ACT table pinned to the Exp set; the
                    # custom-DVE fast reciprocal is broken on this stack)
                    vb = sml.tile([1, 512], bf16, tag="vb", name="vb")
                    nc.vector.tensor_copy(vb[:, :], vare[:, :])
                    nc.vector.tensor_scalar(rs_bf[0:1, fl].bitcast(i16),
                                            vb[:, :].bitcast(i16),
                                            -0.5, KRSQ, ALU.mult, ALU.add)
                    nc.vector.tensor_scalar(srt_row[0:1, fl].bitcast(i16),
                                            vb[:, :].bitcast(i16),
                                            0.5, KSQ, ALU.mult, ALU.add)
                    nc.scalar.copy(rs_row[0:1, fl], rs_bf[0:1, fl])
                    # rs broadcast down 128 rows (bf16 matmul) -> SBUF
                    rsb_ps = lnp.tile([128, 512], f32, tag="rsb", name="rsb")
                    nc.tensor.matmul(rsb_ps[:, :], ones_row[:, :], rs_bf[0:1, fl],
                                     start=True, stop=True)
                    nc.vector.tensor_copy(rs_ball[:, fl], rsb_ps[:, :])
                    # rstd row -> column layout (PE transposes, 4 chunks of 128)
                    for t in range(4):
                        j = f * 4 + t
                        nc.tensor.transpose(rsT_ps[:, j:j + 1],
                                            rs_row[0:1, j * 128:(j + 1) * 128], ident[:, :])
                    nc.vector.tensor_copy(rs_cols[:, f * 4:f * 4 + 4],
                                          rsT_ps[:, f * 4:f * 4 + 4])

                    # K projection for this token chunk (both output halves)
                    for co in range(2):
                        cs = slice(co * 128, (co + 1) * 128)
                        ps = mmp.tile([128, 512], f32, tag="proj", name="proj")
                        for ci in range(2):
                            nc.tensor.matmul(ps[:, :], w_sb["k", ci][:, cs],
                                             xb[ci][:, fl], start=(ci == 0), stop=False)
                        nc.tensor.matmul(ps[:, :], wb_sb["k0"][:, cs],
                                         mu_row[0:1, fl], start=False, stop=False)
                        nc.tensor.matmul(ps[:, :], wb_sb["k1"][:, cs],
                                         srt_row[0:1, fl], start=False, stop=True)
                        nc.vector.tensor_tensor(kT[co][:, fl], ps[:, :], rs_ball[:, fl], ALU.mult)

                    # Q projection (only first two chunks = this core's queries)
                    if f < 2:
                        for co in range(2):
                            cs = slice(co * 128, (co + 1) * 128)
                            ps = mmp.tile([128, 512], f32, tag="proj", name="proj")
                            for ci in range(2):
                                nc.tensor.matmul(ps[:, :], w_sb["q", ci][:, cs],
                                                 xb[ci][:, fl], start=(ci == 0), stop=False)
                            nc.tensor.matmul(ps[:, :], wb_sb["q0"][:, cs],
                                             mu_row[0:1, fl], start=False, stop=False)
                            nc.tensor.matmul(ps[:, :], wb_sb["q1"][:, cs],
                                             srt_row[0:1, fl], start=False, stop=True)
                            for h in range(4):
                                rr = slice(h * 32, (h + 1) * 32)
                                nc.vector.tensor_tensor(qp[co][h][rr, fl], ps[rr, :],
                                                        rs_ball[rr, fl], ALU.mult)

                # V projection per 128-token chunk (tokens in partitions). The
                # 33rd "dummy" channel per head has zero weights and rank-1
                # bias = sqrt(var+eps), so after the *rstd evacuation it is
                # exactly the ones column (softmax denominator accumulator).
                for j in range(32):
                    jl = slice(j * 128, (j + 1) * 128)
                    ps = mmp.tile([128, NH * 33], f32, tag="proj", name="vproj")
                    for ci in range(2):
                        nc.tensor.matmul(ps[:, :], xb[ci][:, jl], w_sb["v", ci][:, :],
                                         start=(ci == 0), stop=False)
                    nc.tensor.matmul(ps[:, :], mu_row[0:1, jl],
                                     wb_sb["v0"][:, :], start=False, stop=False)
                    nc.tensor.matmul(ps[:, :], srt_row[0:1, jl],
                                     wb_sb["v1"][:, :], start=False, stop=True)
                    nc.scalar.mul(v_sb[:, j, :, :],
                                  ps[:, :].rearrange("p (h e) -> p h e", h=NH),
                                  rs_cols[:, j:j + 1])

            # ---- attention ----
            with tc.tile_pool(name="sps", bufs=1, space="PSUM") as sp, \
                 tc.tile_pool(name="avp", bufs=1, space="PSUM") as avp, \
                 tc.tile_pool(name="bcp", bufs=1, space="PSUM") as bcp, \
                 tc.tile_pool(name="pp", bufs=2) as ppool, \
                 tc.tile_pool(name="nrm", bufs=2) as nrm:
                with tc.tile_pool(name="mm2", bufs=1, space="PSUM") as mm2, \
                     tc.tile_pool(name="ot", bufs=4) as otp:
                    for f in range(2):
                        fl = slice(f * 512, (f + 1) * 512)
                        for hg in range(2):
                            av = [avp.tile([128, 512], f32, tag=f"av{pr}", name=f"av{pr}")
                                  for pr in range(2)]
                            for j in range(32):
                                jl = slice(j * 128, (j + 1) * 128)
                                ss = [sp.tile([128, 1024], f32, tag=f"s{i}", name=f"s{i}")
                                      for i in range(2)]
                                pt = [ppool.tile([128, 1024], bf16, tag=f"p{i}", name=f"p{i}")
                                      for i in range(2)]
                                for i in range(2):
                                    for t2 in range(2):
                                        h = i * 2 + t2
                                        nc.tensor.matmul(ss[i][:, t2 * 512:(t2 + 1) * 512],
                                                         kT[hg][:, jl], qp[hg][h][:, fl],
                                                         start=True, stop=True)
                                # pair 0: true exp on ScalarE; pair 1: Schraudolph on VectorE
                                nc.scalar.activation(pt[0][:, :], ss[0][:, :],
                                                     ACTF.Exp, scale=LN2 / 128.0)
                                nc.vector.tensor_scalar(pt[1][:, :].bitcast(i16), ss[1][:, :],
                                                        B16F, None, ALU.add)
                                for pr in range(2):
                                    for t2 in range(2):
                                        h = pr * 2 + t2
                                        nc.tensor.matmul(
                                            av[pr][t2 * 64:t2 * 64 + 33, :],
                                            v_sb[:, j, hg * 4 + h, :],
                                            pt[pr][:, t2 * 512:(t2 + 1) * 512],
                                            start=(j == 0), stop=(j == 31),
                                            tile_position=(0, t2 * 64))
                            # normalization: bit-trick reciprocal of the PSUM
                            # ones-row denominator, broadcast, multiply
                            for pr in range(2):
                                for t2 in range(2):
                                    db = nrm.tile([1, 512], bf16, tag="db", name="db")
                                    nc.vector.tensor_copy(
                                        db[:, :], av[pr][t2 * 64 + 32:t2 * 64 + 33, :])
                                    rcpb = nrm.tile([1, 512], bf16, tag="rb", name="rb")
                                    nc.vector.tensor_scalar(rcpb[:, :].bitcast(i16),
                                                            db[:, :].bitcast(i16),
                                                            -1.0, KRCP, ALU.mult, ALU.add)
                                    bcq = bcp.tile([32, 512], f32, tag="bcq", name="bcq")
                                    nc.tensor.matmul(bcq[:, :], ones_row[:, 0:32],
                                                     rcpb[:, :], start=True, stop=True)
                                    bcs = nrm.tile([32, 512], bf16, tag="bcs", name="bcs")
                                    nc.vector.tensor_copy(bcs[:, :], bcq[:, :])
                                    row0 = (pr * 2 + t2) * 32
                                    nc.vector.tensor_tensor(
                                        attnT[hg][row0:row0 + 32, fl],
                                        av[pr][t2 * 64:t2 * 64 + 32, :],
                                        bcs[:, :], ALU.mult)
                        # output projection + bias + residual for this f-chunk
                        # (overlaps the next attention group's matmuls)
                        for mo in range(2):
                            ms = slice(mo * 128, (mo + 1) * 128)
                            ps = mm2.tile([128, 512], f32, tag="o", name="o")
                            for ci in range(2):
                                nc.tensor.matmul(ps[:, :], w_sb["p", ci][:, ms],
                                                 attnT[ci][:, fl], start=(ci == 0), stop=(ci == 1))
                            ot = otp.tile([128, 512], f32, tag="ot", name="ot")
                            nc.vector.scalar_tensor_tensor(ot[:, :], ps[:, :], bp_sb[mo][:, :],
                                                           xf[mo][:, fl], ALU.add, ALU.add)
                            nc.sync.dma_start(out=od[ms, fl], in_=ot[:, :])

    nc.finalize()
    return nc


def kernel(x, ln_gamma, ln_beta, w_qkv, w_proj, b_proj):
    import ml_dtypes
    from concourse.bass_utils import run_bass_kernel_spmd

    if "nc" not in _CACHE:
        _CACHE["nc"] = _build()
    nc = _CACHE["nc"]

    bf = ml_dtypes.bfloat16
    x = np.asarray(x, np.float32)
    w_qkv = np.asarray(w_qkv, np.float32)
    gam = np.asarray(ln_gamma, np.float32)
    bet = np.asarray(ln_beta, np.float32)
    wq_, wk_, wv_ = w_qkv[0:C], w_qkv[C:2 * C], w_qkv[2 * C:3 * C]

    def prep(wmat, scale):
        wg = (scale * wmat * gam[None, :]).astype(bf)           # [o, c] gamma folded
        wT = np.ascontiguousarray(wg.T)                         # lhsT layout [in, out]
        sw = wg.astype(np.float32).sum(1)                       # rowsum of device weights
        bias = scale * (wmat @ bet)
        return (wT, np.ascontiguousarray(-sw[None, :].astype(bf)),
                np.ascontiguousarray(bias[None, :].astype(bf)))

    wqT, wbq0_h, wbq1_h = prep(wq_, A_SCALE)
    wkT, wbk0_h, wbk1_h = prep(wk_, 1.0)
    # V extended with a zero-weight dummy channel per head whose rank-1 bias
    # is 1 against the srt row (becomes the softmax-denominator ones column).
    wvg = (wv_ * gam[None, :]).astype(bf)
    wv_ext = np.zeros((NH * 33, C), bf)
    wbv0_h = np.zeros((1, NH * 33), np.float32)
    wbv1_h = np.zeros((1, NH * 33), np.float32)
    for h in range(NH):
        wv_ext[h * 33:h * 33 + 32] = wvg[h * 32:(h + 1) * 32]
        wbv0_h[0, h * 33:h * 33 + 32] = -wvg[h * 32:(h + 1) * 32].astype(np.float32).sum(1)
        wbv1_h[0, h * 33:h * 33 + 32] = (wv_ @ bet)[h * 32:(h + 1) * 32]
        wbv1_h[0, h * 33 + 32] = 1.0
    wvT = np.ascontiguousarray(wv_ext.T)
    wbv0_h = wbv0_h.astype(bf)
    wbv1_h = wbv1_h.astype(bf)
    wpT = np.ascontiguousarray(np.asarray(w_proj, np.float32).T.astype(bf))
    bp = np.asarray(b_proj, np.float32).reshape(C, 1)

    xfull = x.reshape(B, C, N)
    in_maps = []
    for core in range(8):
        b, qc = core // 4, core % 4
        xr = np.roll(xfull[b], -qc * NQ, axis=1)
        in_maps.append({
            "xb": np.ascontiguousarray(xr.astype(bf)),
            "xf": np.ascontiguousarray(xr[:, :NQ]),
            "wqT": wqT, "wkT": wkT, "wvT": wvT, "wpT": wpT,
            "wbq0": wbq0_h, "wbq1": wbq1_h, "wbk0": wbk0_h, "wbk1": wbk1_h,
            "wbv0": wbv0_h, "wbv1": wbv1_h, "bp": bp,
        })

    res = run_bass_kernel_spmd(nc, in_maps, core_ids=list(range(8)),
                               trace=_PROFILE)
    if _PROFILE:
        _CACHE["exec_time_ns"] = res.exec_time_ns
    out = np.empty((B, C, N), np.float32)
    for core in range(8):
        b, qc = core // 4, core % 4
        out[b][:, qc * NQ:(qc + 1) * NQ] = res.results[core]["out"]
    return out.reshape(B, C, H, W)


# revision 37
# speedup vs baseline: 1.5098x; 1.0103x over previous
"""Trainium2 Bass kernel for nn_Attention (B=2, C=256, H=W=64, 8 heads).

Sharding: 8 cores = 2 batches x 4 query-chunks (1024 queries each), no
collectives. Each core gets its batch's full x (bf16) with token columns
rolled so its own query chunk sits at columns 0:1024 (attention is
permutation-invariant over keys); it computes LN + projections + attention
for its queries and writes a [256, 1024] slice of the output.

Key structure (v2 — rebuilt for PE throughput; 703us -> 470us):
- x ships as bf16 (attention path + LN stats) plus an f32 query-slice for
  the exact residual.
- LN is folded into the projections: gamma is pre-multiplied into the
  weights on the host; the per-token mean and sqrt(var+eps) enter as two
  rank-1 fixup matmuls accumulated into each projection's PSUM
  (-rowsum(W') x mu and (W@beta) x srt), followed by a *rstd multiply at
  PSUM evacuation. No normalized-x tensor is ever materialized. rstd and
  sqrt come from bf16 exponent bit-tricks on VectorE (the custom-DVE fast
  reciprocal is broken on this stack, and mixing Ln into ScalarE thrashes
  the ACT table against Exp at 2.7us/reload).
- S^T matmuls are full-array (unmasked): the stationary is the whole
  4-head K chunk [128x128]; per-head Q lives in zero-padded [128, NQ]
  tiles so each 512-query matmul contracts over all 128 channel rows but
  only the head's 32 rows are nonzero. This keeps the PE HAM clock-gate
  at 2.4 GHz (masked tile_position matmuls don't count as PE activity —
  the whole attention phase otherwise runs at the cold 1.2 GHz clock).
- exp splits per key-chunk between ScalarE (true exp on head-pair 0) and
  VectorE (Schraudolph bit-trick on head-pair 1): Wq is pre-scaled so
  PSUM = 128*log2(e)*logit; adding a magic bias and converting f32->int16
  yields the bf16 bit pattern of 2^y (~3% max rel err, harmless next to
  the residual). Both pairs process [128,1024] two-bank PSUM tiles.
- P^T V via stationary-V matmuls whose 33rd per-head column is a dummy
  output channel (zero weights, rank-1 bias = srt, so after the *rstd
  evacuation it is exactly 1): the AV accumulation emits the softmax
  denominators for free. Normalization is a bf16 bit-trick reciprocal
  broadcast down 32 rows through a tiny bf16 matmul.
- Per-f output projection (+bias +f32 residual in one fused
  scalar_tensor_tensor) overlaps the next attention group.
"""

import numpy as np

B, C, H, W = 2, 256, 64, 64
N = H * W            # 4096 tokens
NH, HD = 8, 32       # heads, head_dim
NQ = N // 4          # queries per core
LN_EPS = 1e-5
LOG2E = 1.4426950408889634
LN2 = 0.6931471805599453
ATTN_SCALE = HD ** -0.5
A_SCALE = 128.0 * LOG2E * ATTN_SCALE   # folded into Wq on host
B16F = 16256.0 - 5.6                   # Schraudolph bias (calibrated)
KRSQ = 24375.25                        # bf16 bit-trick rsqrt bias (<=3.7% rel)
KSQ = 8123.0                           # bf16 bit-trick sqrt bias (<=3.9% rel)
KRCP = 32498.75                        # bf16 bit-trick recip bias (<=5.3% rel)

_PROFILE = False
_CACHE = {}


def _build():
    from concourse import bacc
    from concourse import mybir
    import concourse.tile as tile

    f32 = mybir.dt.float32
    f32r = mybir.dt.float32r
    bf16 = mybir.dt.bfloat16
    i16 = mybir.dt.int16
    ALU = mybir.AluOpType
    ACTF = mybir.ActivationFunctionType

    nc = bacc.Bacc("TRN2", target_bir_lowering=False)
    xbd = nc.dram_tensor("xb", [C, N], bf16, kind="ExternalInput")
    xfd = nc.dram_tensor("xf", [C, NQ], f32, kind="ExternalInput")
    wq = nc.dram_tensor("wqT", [C, C], bf16, kind="ExternalInput")  # gamma+A_SCALE folded
    wk = nc.dram_tensor("wkT", [C, C], bf16, kind="ExternalInput")
    wv = nc.dram_tensor("wvT", [C, NH * 33], bf16, kind="ExternalInput")
    wp = nc.dram_tensor("wpT", [C, C], bf16, kind="ExternalInput")
    # rank-1 LN fixup rows: *0 = -rowsum(W'), *1 = W@beta (each own tensor so
    # every engine/matmul access starts at partition 0)
    wbq0 = nc.dram_tensor("wbq0", [1, C], bf16, kind="ExternalInput")
    wbq1 = nc.dram_tensor("wbq1", [1, C], bf16, kind="ExternalInput")
    wbk0 = nc.dram_tensor("wbk0", [1, C], bf16, kind="ExternalInput")
    wbk1 = nc.dram_tensor("wbk1", [1, C], bf16, kind="ExternalInput")
    wbv0 = nc.dram_tensor("wbv0", [1, NH * 33], bf16, kind="ExternalInput")
    wbv1 = nc.dram_tensor("wbv1", [1, NH * 33], bf16, kind="ExternalInput")
    bpd = nc.dram_tensor("bp", [C, 1], f32, kind="ExternalInput")
    od = nc.dram_tensor("out", [C, NQ], f32, kind="ExternalOutput")

    with tile.TileContext(nc) as tc:
        with tc.tile_pool(name="big", bufs=1) as big, \
             tc.tile_pool(name="sml", bufs=2) as sml:

            # ---- load inputs ----
            xb = [big.tile([128, N], bf16, tag=f"xb{c}", name=f"xb{c}") for c in range(2)]
            for q4 in range(4):
                qs = slice(q4 * 1024, (q4 + 1) * 1024)
                for c in range(2):
                    nc.sync.dma_start(out=xb[c][:, qs], in_=xbd[c * 128:(c + 1) * 128, qs])
            xf = [big.tile([128, NQ], f32, tag=f"xf{c}", name=f"xf{c}") for c in range(2)]
            for c in range(2):
                nc.sync.dma_start(out=xf[c][:, :], in_=xfd[c * 128:(c + 1) * 128, :])
            w_sb = {}
            for name, t, nout in (("q", wq, C), ("k", wk, C),
                                  ("v", wv, NH * 33), ("p", wp, C)):
                for c in range(2):
                    s = big.tile([128, nout], bf16, tag=f"w{name}{c}", name=f"w{name}{c}")
                    nc.sync.dma_start(out=s[:, :], in_=t[c * 128:(c + 1) * 128, :])
                    w_sb[name, c] = s
            wb_sb = {}
            for name, t, nout in (("q0", wbq0, C), ("q1", wbq1, C),
                                  ("k0", wbk0, C), ("k1", wbk1, C),
                                  ("v0", wbv0, NH * 33), ("v1", wbv1, NH * 33)):
                s = big.tile([1, nout], bf16, tag=f"wb{name}", name=f"wb{name}")
                nc.sync.dma_start(out=s[:, :], in_=t[:, :])
                wb_sb[name] = s
            bp_sb = [big.tile([128, 1], f32, tag=f"bp{c}", name=f"bp{c}") for c in range(2)]
            for c in range(2):
                nc.sync.dma_start(out=bp_sb[c][:, :], in_=bpd[c * 128:(c + 1) * 128, :])

            onesC = big.tile([128, 1], bf16, tag="onesC", name="onesC")
            nc.vector.memset(onesC[:, :], 1.0 / C)
            ones_row = big.tile([1, 128], bf16, tag="onesr", name="onesr")
            nc.vector.memset(ones_row[:, :], 1.0)
            ident = big.tile([1, 1], f32, tag="ident", name="ident")
            nc.vector.memset(ident[:, :], 1.0)

            rs_cols = big.tile([128, 32], f32, tag="rscols", name="rscols")  # rstd, col layout

            kT = [big.tile([128, N], bf16, tag=f"kT{c}", name=f"kT{c}") for c in range(2)]
            qp = [[big.tile([128, NQ], bf16, tag=f"qp{hg}{h}", name=f"qp{hg}{h}")
                   for h in range(4)] for hg in range(2)]
            for hg in range(2):
                for h in range(4):
                    nc.gpsimd.memset(qp[hg][h][:, :], 0.0)
            v_sb = big.tile([128, 32, NH, 33], bf16, tag="v", name="v")
            attnT = [big.tile([128, NQ], bf16, tag=f"at{c}", name=f"at{c}") for c in range(2)]

            # ---- LN stats + projections ----
            with tc.tile_pool(name="lnsb", bufs=1) as lnsb, \
                 tc.tile_pool(name="lnp", bufs=1, space="PSUM") as lnp, \
                 tc.tile_pool(name="mm", bufs=2, space="PSUM") as mmp:
                mu_row = lnsb.tile([1, N], bf16, tag="murow", name="murow")
                srt_row = lnsb.tile([1, N], bf16, tag="srtrow", name="srtrow")
                rs_row = lnsb.tile([1, N], f32, tag="rsrow", name="rsrow")
                rs_bf = lnsb.tile([1, N], bf16, tag="rsbf", name="rsbf")
                rs_ball = lnsb.tile([128, N], f32, tag="rsball", name="rsball")
                xsq = [lnsb.tile([128, N], bf16, tag=f"xsq{c}", name=f"xsq{c}") for c in range(2)]
                nc.vector.tensor_tensor(xsq[0][:, :], xb[0][:, :], xb[0][:, :], ALU.mult)
                nc.vector.tensor_tensor(xsq[1][:, :], xb[1][:, :], xb[1][:, :], ALU.mult)
                rsT_ps = lnp.tile([128, 32], f32, tag="rsT", name="rsT")
                for f in range(8):
                    fl = slice(f * 512, (f + 1) * 512)
                    mps = lnp.tile([1, 512], f32, tag="mps", name="mps")
                    nc.tensor.matmul(mps[:, :], onesC[:, :], xb[0][:, fl], start=True, stop=False)
                    nc.tensor.matmul(mps[:, :], onesC[:, :], xb[1][:, fl], start=False, stop=True)
                    sps = lnp.tile([1, 512], f32, tag="sps", name="sps")
                    nc.tensor.matmul(sps[:, :], onesC[:, :], xsq[0][:, fl], start=True, stop=False)
                    nc.tensor.matmul(sps[:, :], onesC[:, :], xsq[1][:, fl], start=False, stop=True)
                    # mu row (SBUF, bf16) + vare = (msq + eps) - mu^2
                    nc.scalar.copy(mu_row[0:1, fl], mps[:, :])
                    mu2 = sml.tile([1, 512], f32, tag="mu2", name="mu2")
                    nc.vector.tensor_tensor(mu2[:, :], mu_row[0:1, fl], mu_row[0:1, fl], ALU.mult)
                    vare = sml.tile([1, 512], f32, tag="vare", name="vare")
                    nc.vector.scalar_tensor_tensor(vare[:, :], sps[:, :], LN_EPS, mu2[:, :],
                                                   ALU.add, ALU.subtract)
                    # rstd / sqrt via bf16 exponent bit tricks on VectorE
                    # (keeps ScalarE's ACT table pinned to the Exp set; the
                    # custom-DVE fast reciprocal is broken on this stack)
                    vb = sml.tile([1, 512], bf16, tag="vb", name="vb")
                    nc.vector.tensor_copy(vb[:, :], vare[:, :])
                    nc.vector.tensor_scalar(rs_bf[0:1, fl].bitcast(i16),
                                            vb[:, :].bitcast(i16),
                                            -0.5, KRSQ, ALU.mult, ALU.add)
                    nc.vector.tensor_scalar(srt_row[0:1, fl].bitcast(i16),
                                            vb[:, :].bitcast(i16),
                                            0.5, KSQ, ALU.mult, ALU.add)
                    nc.scalar.copy(rs_row[0:1, fl], rs_bf[0:1, fl])
                    # rs broadcast down 128 rows (bf16 matmul) -> SBUF
                    rsb_ps = lnp.tile([128, 512], f32, tag="rsb", name="rsb")
                    nc.tensor.matmul(rsb_ps[:, :], ones_row[:, :], rs_bf[0:1, fl],
                                     start=True, stop=True)
                    nc.vector.tensor_copy(rs_ball[:, fl], rsb_ps[:, :])
                    # rstd row -> column layout (PE transposes, 4 chunks of 128)
                    for t in range(4):
                        j = f * 4 + t
                        nc.tensor.transpose(rsT_ps[:, j:j + 1],
                                            rs_row[0:1, j * 128:(j + 1) * 128], ident[:, :])
                    nc.vector.tensor_copy(rs_cols[:, f * 4:f * 4 + 4],
                                          rsT_ps[:, f * 4:f * 4 + 4])

                    # K projection for this token chunk (both output halves)
                    for co in range(2):
                        cs = slice(co * 128, (co + 1) * 128)
                        ps = mmp.tile([128, 512], f32, tag="proj", name="proj")
                        for ci in range(2):
                            nc.tensor.matmul(ps[:, :], w_sb["k", ci][:, cs],
                                             xb[ci][:, fl], start=(ci == 0), stop=False)
                        nc.tensor.matmul(ps[:, :], wb_sb["k0"][:, cs],
                                         mu_row[0:1, fl], start=False, stop=False)
                        nc.tensor.matmul(ps[:, :], wb_sb["k1"][:, cs],
                                         srt_row[0:1, fl], start=False, stop=True)
                        nc.vector.tensor_tensor(kT[co][:, fl], ps[:, :], rs_ball[:, fl], ALU.mult)

                    # Q projection (only first two chunks = this core's queries)
                    if f < 2:
                        for co in range(2):
                            cs = slice(co * 128, (co + 1) * 128)
                            ps = mmp.tile([128, 512], f32, tag="proj", name="proj")
                            for ci in range(2):
                                nc.tensor.matmul(ps[:, :], w_sb["q", ci][:, cs],
                                                 xb[ci][:, fl], start=(ci == 0), stop=False)
                            nc.tensor.matmul(ps[:, :], wb_sb["q0"][:, cs],
                                             mu_row[0:1, fl], start=False, stop=False)
                            nc.tensor.matmul(ps[:, :], wb_sb["q1"][:, cs],
                                             srt_row[0:1, fl], start=False, stop=True)
                            for h in range(4):
                                rr = slice(h * 32, (h + 1) * 32)
                                nc.vector.tensor_tensor(qp[co][h][rr, fl], ps[rr, :],
                                                        rs_ball[rr, fl], ALU.mult)

                # V projection per 128-token chunk (tokens in partitions). The
                # 33rd "dummy" channel per head has zero weights and rank-1
                # bias = sqrt(var+eps), so after the *rstd evacuation it is
                # exactly the ones column (softmax denominator accumulator).
                for j in range(32):
                    jl = slice(j * 128, (j + 1) * 128)
                    ps = mmp.tile([128, NH * 33], f32, tag="proj", name="vproj")
                    for ci in range(2):
                        nc.tensor.matmul(ps[:, :], xb[ci][:, jl], w_sb["v", ci][:, :],
                                         start=(ci == 0), stop=False)
                    nc.tensor.matmul(ps[:, :], mu_row[0:1, jl],
                                     wb_sb["v0"][:, :], start=False, stop=False)
                    nc.tensor.matmul(ps[:, :], srt_row[0:1, jl],
                                     wb_sb["v1"][:, :], start=False, stop=True)
                    nc.scalar.mul(v_sb[:, j, :, :],
                                  ps[:, :].rearrange("p (h e) -> p h e", h=NH),
                                  rs_cols[:, j:j + 1])

            # ---- attention ----
            with tc.tile_pool(name="sps", bufs=1, space="PSUM") as sp, \
                 tc.tile_pool(name="avp", bufs=1, space="PSUM") as avp, \
                 tc.tile_pool(name="bcp", bufs=1, space="PSUM") as bcp, \
                 tc.tile_pool(name="pp", bufs=2) as ppool, \
                 tc.tile_pool(name="nrm", bufs=2) as nrm:
                with tc.tile_pool(name="mm2", bufs=1, space="PSUM") as mm2, \
                     tc.tile_pool(name="ot", bufs=4) as otp:
                    for f in range(2):
                        fl = slice(f * 512, (f + 1) * 512)
                        for hg in range(2):
                            av = [avp.tile([128, 512], f32, tag=f"av{pr}", name=f"av{pr}")
                                  for pr in range(2)]
                            for j in range(32):
                                jl = slice(j * 128, (j + 1) * 128)
                                ss = [sp.tile([128, 1024], f32, tag=f"s{i}", name=f"s{i}")
                                      for i in range(2)]
                                pt = [ppool.tile([128, 1024], bf16, tag=f"p{i}", name=f"p{i}")
                                      for i in range(2)]
                                for i in range(2):
                                    for t2 in range(2):
                                        h = i * 2 + t2
                                        nc.tensor.matmul(ss[i][:, t2 * 512:(t2 + 1) * 512],
                                                         kT[hg][:, jl], qp[hg][h][:, fl],
                                                         start=True, stop=True)
                                # pair 0: true exp on ScalarE; pair 1: Schraudolph on VectorE
                                nc.scalar.activation(pt[0][:, :], ss[0][:, :],
                                                     ACTF.Exp, scale=LN2 / 128.0)
                                nc.vector.tensor_scalar(pt[1][:, :].bitcast(i16), ss[1][:, :],
                                                        B16F, None, ALU.add)
                                for pr in range(2):
                                    for t2 in range(2):
                                        h = pr * 2 + t2
                                        nc.tensor.matmul(
                                            av[pr][t2 * 64:t2 * 64 + 33, :],
                                            v_sb[:, j, hg * 4 + h, :],
                                            pt[pr][:, t2 * 512:(t2 + 1) * 512],
                                            start=(j == 0), stop=(j == 31),
                                            tile_position=(0, t2 * 64))
                            # normalization: bit-trick reciprocal of the PSUM
                            # ones-row denominator, broadcast, multiply
                            for pr in range(2):
                                for t2 in range(2):
                                    db = nrm.tile([1, 512], bf16, tag="db", name="db")
                                    nc.vector.tensor_copy(
                                        db[:, :], av[pr][t2 * 64 + 32:t2 * 64 + 33, :])
                                    rcpb = nrm.tile([1, 512], bf16, tag="rb", name="rb")
                                    nc.vector.tensor_scalar(rcpb[:, :].bitcast(i16),
                                                            db[:, :].bitcast(i16),
                                                            -1.0, KRCP, ALU.mult, ALU.add)
                                    bcq = bcp.tile([32, 512], f32, tag="bcq", name="bcq")
                                    nc.tensor.matmul(bcq[:, :], ones_row[:, 0:32],
                                                     rcpb[:, :], start=True, stop=True)
                                    bcs = nrm.tile([32, 512], bf16, tag="bcs", name="bcs")
                                    nc.vector.tensor_copy(bcs[:, :], bcq[:, :])
                                    row0 = (pr * 2 + t2) * 32
                                    nc.vector.tensor_tensor(
                                        attnT[hg][row0:row0 + 32, fl],
                                        av[pr][t2 * 64:t2 * 64 + 32, :],
                                        bcs[:, :], ALU.mult)
                        # output projection + bias + residual for this f-chunk
                        # (overlaps the next attention group's matmuls)
                        for mo in range(2):
                            ms = slice(mo * 128, (mo + 1) * 128)
                            ps = mm2.tile([128, 512], f32, tag="o", name="o")
                            for ci in range(2):
                                nc.tensor.matmul(ps[:, :], w_sb["p", ci][:, ms],
                                                 attnT[ci][:, fl], start=(ci == 0), stop=(ci == 1))
                            ot = otp.tile([128, 512], f32, tag="ot", name="ot")
                            nc.vector.scalar_tensor_tensor(ot[:, :], ps[:, :], bp_sb[mo][:, :],
                                                           xf[mo][:, fl], ALU.add, ALU.add)
                            nc.sync.dma_start(out=od[ms, fl], in_=ot[:, :])

    nc.finalize()
    return nc


def kernel(x, ln_gamma, ln_beta, w_qkv, w_proj, b_proj):
    import ml_dtypes
    from concourse.bass_utils import run_bass_kernel_spmd

    if "nc" not in _CACHE:
        _CACHE["nc"] = _build()
    nc = _CACHE["nc"]

    bf = ml_dtypes.bfloat16
    x = np.asarray(x, np.float32)
    w_qkv = np.asarray(w_qkv, np.float32)
    gam = np.asarray(ln_gamma, np.float32)
    bet = np.asarray(ln_beta, np.float32)
    wq_, wk_, wv_ = w_qkv[0:C], w_qkv[C:2 * C], w_qkv[2 * C:3 * C]

    def prep(wmat, scale):
        wg = (scale * wmat * gam[None, :]).astype(bf)           # [o, c] gamma folded
        wT = np.ascontiguousarray(wg.T)                         # lhsT layout [in, out]
        sw = wg.astype(np.float32).sum(1)                       # rowsum of device weights
        bias = scale * (wmat @ bet)
        return (wT, np.ascontiguousarray(-sw[None, :].astype(bf)),
                np.ascontiguousarray(bias[None, :].astype(bf)))

    wqT, wbq0_h, wbq1_h = prep(wq_, A_SCALE)
    wkT, wbk0_h, wbk1_h = prep(wk_, 1.0)
    # V extended with a zero-weight dummy channel per head whose rank-1 bias
    # is 1 against the srt row (becomes the softmax-denominator ones column).
    wvg = (wv_ * gam[None, :]).astype(bf)
    wv_ext = np.zeros((NH * 33, C), bf)
    wbv0_h = np.zeros((1, NH * 33), np.float32)
    wbv1_h = np.zeros((1, NH * 33), np.float32)
    for h in range(NH):
        wv_ext[h * 33:h * 33 + 32] = wvg[h * 32:(h + 1) * 32]
        wbv0_h[0, h * 33:h * 33 + 32] = -wvg[h * 32:(h + 1) * 32].astype(np.float32).sum(1)
        wbv1_h[0, h * 33:h * 33 + 32] = (wv_ @ bet)[h * 32:(h + 1) * 32]
        wbv1_h[0, h * 33 + 32] = 1.0
    wvT = np.ascontiguousarray(wv_ext.T)
    wbv0_h = wbv0_h.astype(bf)
    wbv1_h = wbv1_h.astype(bf)
    wpT = np.ascontiguousarray(np.asarray(w_proj, np.float32).T.astype(bf))
    bp = np.asarray(b_proj, np.float32).reshape(C, 1)

    xfull = x.reshape(B, C, N)
    in_maps = []
    for core in range(8):
        b, qc = core // 4, core % 4
        xr = np.roll(xfull[b], -qc * NQ, axis=1)
        in_maps.append({
            "xb": np.ascontiguousarray(xr.astype(bf)),
            "xf": np.ascontiguousarray(xr[:, :NQ]),
            "wqT": wqT, "wkT": wkT, "wvT": wvT, "wpT": wpT,
            "wbq0": wbq0_h, "wbq1": wbq1_h, "wbk0": wbk0_h, "wbk1": wbk1_h,
            "wbv0": wbv0_h, "wbv1": wbv1_h, "bp": bp,
        })

    res = run_bass_kernel_spmd(nc, in_maps, core_ids=list(range(8)),
                               trace=_PROFILE)
    if _PROFILE:
        _CACHE["exec_time_ns"] = res.exec_time_ns
    out = np.empty((B, C, N), np.float32)
    for core in range(8):
        b, qc = core // 4, core % 4
        out[b][:, qc * NQ:(qc + 1) * NQ] = res.results[core]["out"]
    return out.reshape(B, C, H, W)
